# revision 47
# baseline (speedup 1.0000x reference)
"""Trainium2 Bass kernel for batched 8-link serial-chain forward kinematics.

Problem: for each batch element b with joint angles q[b, 0:8]:
    T_0 = I4
    T_i = T_{i-1} @ [[Rf_i, tf_i],[0,1]] @ [[Rj(q_i), 0],[0,1]]
    output[b] = stack(T_0 .. T_8)  -> [B, 9, 4, 4] float32

Current default variant "v4" (same-day HW A/B: 115.5us vs v3's 252.7us,
i.e. ~2.2x faster; the absolute number varies ~1.8x with the axon device
state, so compare variants same-day same-method only). Knobs x=dve,
y_act=8, tu_act=1, pool_cols=48: moving the 114 small TS instructions
(Y=s*B2 and the t-chain u_n=Rp*tf products) from DVE to ACT bought 19%
on HW (~0.25us/instr net, far more than the cost model predicts —
DVE per-instruction overhead on silicon is the dominant unmodeled cost;
prefer fewer/bigger DVE ops and park small ops on ACT):
  - all-fp16 compute in compact component-major tiles [P, comp, elem]
    (elem innermost unit-stride -> DVE 2x TT / 4x TS perf modes; verified
    on HW: indep fp16 TT [9,256] = 1301ns, chained +490ns RAW stall,
    fp32 TT = 2525ns);
  - only the 93 non-constant output components are written to HBM as
    fp16 [P, NL, 12, ept] (3 batched fat DMAs, 24KB contiguous per
    partition); the host fills I4/bottom-row/t_1 constants and casts;
  - ACT: trig (Sin table is ONLY valid on [-pi,pi]: 1e-7 err inside,
    garbage beyond |x|>4 -> both range-wraps are mandatory) + X=c*B1+B0;
  - DVE: wraps, Y=s*B2 (TS), Rl=X+Y, 3x3 chain products+adds, t-chain;
  - Pool: an independent sub-chain replaying the same algorithm on the
    last 48 element-columns (element-split data parallelism);
  - emission is software-pipelined: Y/Rl built 2 links ahead, t-chain
    scalar products interleaved between dependent chain ops;
  - timing methodology: per-dispatch tunnel latency is ~70-100ms(!), so
    test.py benches wall(For_i x199 in-NEFF) vs wall(1) — the only
    reliable method here (old slope-minus-trivial method gives garbage).

Older variant "v3" (fp32 element-major, recorded 142us):
  - Each core handles B/8 = 32768 elements: 128 partitions x 256
    elements/partition, processed in column-chunks (large..small so the
    final output DMA tail is short), double-buffered assembly tiles.
  - Rodrigues rotation as a linear form: Rl_i = B0_i + cos(q_i)*B1_i +
    sin(q_i)*B2_i with per-link constant 3x3 matrices (precomputed on host,
    baked into the instruction stream as immediates).
  - cos/sin on ScalarE Sin spline (inputs range-wrapped into [-pi,pi] by the
    custom add_range_wrap DVE op; cos = sin(q + pi/2)).
  - Rl construction off the critical engine: affine c*B1+B0 on ScalarE
    (activation Copy with scale/bias immediates), s*B2 via a broadcast
    tensor_tensor against a replicated B2 constants tile plus the final add
    on GPSIMD.
  - 3x3 chain products on VectorE: per-k merged multi-dim-AP tensor_tensor
    (broadcast dims; <=3 free dims per AP, a hardware ISA limit), reading
    R_{L-1} strided straight out of the assembly tile.
  - Translation chain: 3 fused scalar_tensor_tensor ops on VectorE.
  - Link 1 runs entirely on VectorE (tensor_scalar + scalar_tensor_tensor)
    to fill the chunk-start pipeline bubble.
  - Results are written directly (strided APs) into an element-major
    assembly tile [128, U, 144]; constant positions (T_0 = I4, bottom rows,
    t_1 = tf_1) are memset once per buffer and never rewritten. One big
    contiguous DMA per chunk writes U*144 floats per partition to HBM.
  - A "v3fp16" variant (chain state in fp16 for DVE 2x mode) exists but is
    not the default: it reaches ~3e-3 max abs error vs the fp32 reference.
"""

import numpy as np

_P = 128          # SBUF partitions
_NL = 8           # moving links
_B_FULL = 262144  # full batch
_N_CORES = 8
_U = 128          # elements per partition per chunk

_PI = float(np.pi)


def _fk_constants(fixed_rot, fixed_trans, joint_axis):
    """Per-link constants: Rl_i(q) = B0 + cos(q) B1 + sin(q) B2."""
    fr = np.asarray(fixed_rot, dtype=np.float64)     # [8,3,3]
    ax = np.asarray(joint_axis, dtype=np.float64)    # [8,3]
    tf = np.asarray(fixed_trans, dtype=np.float64)   # [8,3]
    B0 = np.zeros((_NL, 3, 3)); B1 = np.zeros((_NL, 3, 3)); B2 = np.zeros((_NL, 3, 3))
    I3 = np.eye(3)
    for i in range(_NL):
        a = ax[i]
        K = np.array([[0, -a[2], a[1]], [a[2], 0, -a[0]], [-a[1], a[0], 0]])
        aa = np.outer(a, a)
        B0[i] = fr[i] @ aa
        B1[i] = fr[i] @ (I3 - aa)
        B2[i] = fr[i] @ K
    return (B0.astype(np.float32), B1.astype(np.float32), B2.astype(np.float32),
            tf.astype(np.float32))


def _ap(base, extra_off, dims):
    """Build a custom AP on the same tensor as `base` (an AP), keeping its
    partition dim. dims = list of [step, count] in elements."""
    import concourse.bass as bass
    return bass.AP(tensor=base.tensor, offset=base.offset + extra_off,
                   ap=[list(base.ap[0])] + [list(d) for d in dims])


def trace_fk(tc, out_ap, q_ap, B0, B1, B2, TF, b_core, chunk):
    """Trace the per-core FK program into TileContext `tc`.

    out_ap: DRAM [b_core, 9, 4, 4] f32;  q_ap: DRAM [b_core, 8] f32.
    """
    import concourse.mybir as mybir
    from contextlib import ExitStack

    nc = tc.nc
    f32 = mybir.dt.float32
    MULT = mybir.AluOpType.mult
    ADD = mybir.AluOpType.add
    COPY = mybir.ActivationFunctionType.Copy
    SIN = mybir.ActivationFunctionType.Sin

    ept = b_core // _P              # elements per partition (total)
    U = chunk
    nchunks = ept // U
    assert ept % U == 0

    q_view = q_ap.rearrange("(p e) l -> p e l", p=_P)            # [P, ept, 8]
    out_view = out_ap.rearrange("(p e) n r c -> p e (n r c)", p=_P)  # [P, ept, 144]

    with ExitStack() as ctx:
        persist = ctx.enter_context(tc.tile_pool(name="persist", bufs=1))
        scr = ctx.enter_context(tc.tile_pool(name="scr", bufs=1))
        work = scr

        # --- persistent tiles ---
        q_sb = persist.tile([_P, ept, 8], f32)
        nc.sync.dma_start(out=q_sb, in_=q_view)

        asm_tiles = []
        for bi in range(min(2, nchunks)):
            asm = persist.tile([_P, U, 144], f32, tag=f"asm{bi}")
            a = asm[:]
            # T_0 = eye(4): zero the 16-float block, then ones on the diag.
            nc.vector.memset(_ap(a, 0, [[144, U], [1, 16]]), 0.0)
            nc.vector.memset(_ap(a, 0, [[144, U], [5, 4]]), 1.0)
            # bottom rows of T_1..T_8: [0,0,0,1]
            nc.gpsimd.memset(_ap(a, 16 + 12, [[144, U], [16, _NL], [1, 3]]), 0.0)
            nc.gpsimd.memset(_ap(a, 16 + 15, [[144, U], [16, _NL]]), 1.0)
            # t_1 = tf_1 (constant: R_0 = I, t_0 = 0)
            for m in range(3):
                nc.gpsimd.memset(_ap(a, 16 + 4 * m + 3, [[144, U]]), float(TF[0, m]))
            asm_tiles.append(asm)

        for c in range(nchunks):
            asm = asm_tiles[c % 2]
            a = asm[:]
            sl = slice(c * U, (c + 1) * U)

            # ---- angles: wrap + trig, [P, 8, U] link-major ----
            qs_t = work.tile([_P, _NL, U], f32, tag="X")
            qc_t = work.tile([_P, _NL, U], f32, tag="Rl")
            q_in = q_view  # noqa
            q_chunk = q_sb[:, sl, :].rearrange("p j l -> p l j")
            nc.vector.add_range_wrap(out=qs_t[:], in_=q_chunk, shift=0.0,
                                     bound=_PI, period=2 * _PI)
            nc.vector.add_range_wrap(out=qc_t[:], in_=q_chunk, shift=_PI / 2,
                                     bound=_PI, period=2 * _PI)
            s_t = work.tile([_P, _NL, U], f32, tag="s")
            c_t = work.tile([_P, _NL, U], f32, tag="c")
            nc.scalar.activation(out=s_t[:], in_=qs_t[:], func=SIN)
            nc.scalar.activation(out=c_t[:], in_=qc_t[:], func=SIN)

            for L in range(1, _NL + 1):
                i = L - 1           # input link row
                boff = L * 16       # output T-block offset in asm
                poff = (L - 1) * 16  # previous T-block

                # ---- X = c*B1 + B0 (ScalarE), Rl = X + s*B2 (GPSIMD) ----
                X = scr.tile([_P, 9, U], f32, tag="X")
                for mn in range(9):
                    m, n = divmod(mn, 3)
                    nc.gpsimd.tensor_scalar(
                        out=X[:, mn, :], in0=c_t[:, i, :],
                        scalar1=float(B1[i, m, n]), scalar2=float(B0[i, m, n]),
                        op0=MULT, op1=ADD)
                if L == 1:
                    # R_1 = Rl_1 directly into asm (R_0 = I): strided out
                    for mn in range(9):
                        m, n = divmod(mn, 3)
                        nc.vector.scalar_tensor_tensor(
                            out=_ap(a, boff + 4 * m + n, [[144, U]]),
                            in0=s_t[:, i, :], scalar=float(B2[i, m, n]),
                            in1=X[:, mn, :], op0=MULT, op1=ADD)
                    continue

                Rl = scr.tile([_P, 9, U], f32, tag="Rl")
                for mn in range(9):
                    m, n = divmod(mn, 3)
                    nc.vector.scalar_tensor_tensor(
                        out=Rl[:, mn, :], in0=s_t[:, i, :], scalar=float(B2[i, m, n]),
                        in1=X[:, mn, :], op0=MULT, op1=ADD)

                # ---- t chain (GPSIMD): t_L = R_{L-1} tf_L + t_{L-1} ----
                # merged over m: in0 = R_{L-1}[:, k] col (strided from asm)
                acc1 = scr.tile([_P, 3, U], f32, tag="acc1")
                acc2 = scr.tile([_P, 3, U], f32, tag="acc2")
                nc.vector.scalar_tensor_tensor(
                    out=acc1[:], in0=_ap(a, poff + 0, [[4, 3], [144, U]]),
                    scalar=float(TF[i, 0]),
                    in1=_ap(a, poff + 3, [[4, 3], [144, U]]), op0=MULT, op1=ADD)
                nc.vector.scalar_tensor_tensor(
                    out=acc2[:], in0=_ap(a, poff + 1, [[4, 3], [144, U]]),
                    scalar=float(TF[i, 1]), in1=acc1[:], op0=MULT, op1=ADD)
                nc.vector.scalar_tensor_tensor(
                    out=_ap(a, boff + 3, [[4, 3], [144, U]]),
                    in0=_ap(a, poff + 2, [[4, 3], [144, U]]),
                    scalar=float(TF[i, 2]), in1=acc2[:], op0=MULT, op1=ADD)

                # ---- R chain (VectorE): R_L = R_{L-1} @ Rl ----
                # mk: s9[m,n] = Rp[m,k] * Rl[k,n]  (broadcast over n / m)
                s9a = scr.tile([_P, 3, 3, U], f32, tag="s9a")
                s9b = scr.tile([_P, 3, 3, U], f32, tag="s9b")
                s9c = scr.tile([_P, 3, 3, U], f32, tag="s9c")

                def rp_k(k):
                    return _ap(a, poff + k, [[4, 3], [0, 3], [144, U]])

                def rl_k(k):
                    return Rl[:, 3 * k:3 * k + 3, :].unsqueeze(1).broadcast_to(
                        [_P, 3, 3, U])

                nc.vector.tensor_tensor(out=s9a[:], in0=rp_k(0), in1=rl_k(0), op=MULT)
                nc.vector.tensor_tensor(out=s9b[:], in0=rp_k(1), in1=rl_k(1), op=MULT)
                nc.vector.tensor_tensor(out=s9b[:], in0=s9a[:], in1=s9b[:], op=ADD)
                nc.vector.tensor_tensor(out=s9c[:], in0=rp_k(2), in1=rl_k(2), op=MULT)
                nc.vector.tensor_tensor(
                    out=_ap(a, boff, [[4, 3], [1, 3], [144, U]]),
                    in0=s9b[:], in1=s9c[:], op=ADD)

            # ---- store chunk ----
            nc.sync.dma_start(out=out_view[:, sl, :], in_=asm)


def trace_fk_v2(tc, out_ap, q_ap, b2c_ap, B0, B1, B2, TF, b_core, chunk):
    """v2: Rl construction fully on GPSIMD (affine tensor_scalar + broadcast
    tensor_tensor with a B2 constants tile); chain mult merged into one
    tensor_tensor per link on VectorE."""
    import concourse.bass as bass
    import concourse.mybir as mybir
    from contextlib import ExitStack

    nc = tc.nc
    f32 = mybir.dt.float32
    MULT = mybir.AluOpType.mult
    ADD = mybir.AluOpType.add
    SIN = mybir.ActivationFunctionType.Sin

    ept = b_core // _P
    U = chunk
    nchunks = ept // U
    assert ept % U == 0

    q_view = q_ap.rearrange("(p e) l -> p e l", p=_P)
    out_view = out_ap.rearrange("(p e) n r c -> p e (n r c)", p=_P)

    with ExitStack() as ctx:
        persist = ctx.enter_context(tc.tile_pool(name="persist", bufs=1))
        scr = ctx.enter_context(tc.tile_pool(name="scr", bufs=1))
        work = scr

        q_sb = persist.tile([_P, ept, 8], f32)
        nc.sync.dma_start(out=q_sb, in_=q_view)
        # B2 constants replicated across partitions: [P, 8, 9]
        b2c = persist.tile([_P, _NL, 9], f32)
        b2_bcast = bass.AP(tensor=b2c_ap.tensor, offset=b2c_ap.offset,
                           ap=[[0, _P], [9, _NL], [1, 9]])
        nc.sync.dma_start(out=b2c, in_=b2_bcast)

        asm_tiles = []
        for bi in range(min(2, nchunks)):
            asm = persist.tile([_P, U, 144], f32, tag=f"asm{bi}")
            a = asm[:]
            nc.vector.memset(_ap(a, 0, [[144, U], [1, 16]]), 0.0)
            nc.vector.memset(_ap(a, 0, [[144, U], [5, 4]]), 1.0)
            nc.gpsimd.memset(_ap(a, 16 + 12, [[144, U], [16, _NL], [1, 3]]), 0.0)
            nc.gpsimd.memset(_ap(a, 16 + 15, [[144, U], [16, _NL]]), 1.0)
            for m in range(3):
                nc.gpsimd.memset(_ap(a, 16 + 4 * m + 3, [[144, U]]), float(TF[0, m]))
            asm_tiles.append(asm)

        for c in range(nchunks):
            asm = asm_tiles[c % 2]
            a = asm[:]
            sl = slice(c * U, (c + 1) * U)

            qs_t = work.tile([_P, _NL, U], f32, tag="X")
            qc_t = work.tile([_P, _NL, U], f32, tag="Y")
            q_chunk = q_sb[:, sl, :].rearrange("p j l -> p l j")
            nc.vector.add_range_wrap(out=qs_t[:], in_=q_chunk, shift=0.0,
                                     bound=_PI, period=2 * _PI)
            nc.vector.add_range_wrap(out=qc_t[:], in_=q_chunk, shift=_PI / 2,
                                     bound=_PI, period=2 * _PI)
            s_t = work.tile([_P, _NL, U], f32, tag="s")
            c_t = work.tile([_P, _NL, U], f32, tag="c")
            nc.scalar.activation(out=s_t[:], in_=qs_t[:], func=SIN)
            nc.scalar.activation(out=c_t[:], in_=qc_t[:], func=SIN)

            for L in range(1, _NL + 1):
                i = L - 1
                boff = L * 16
                poff = (L - 1) * 16

                # ---- Rl on GPSIMD ----
                X = scr.tile([_P, 9, U], f32, tag="X")
                Y = scr.tile([_P, 9, U], f32, tag="Y")
                for mn in range(9):
                    m, n = divmod(mn, 3)
                    nc.gpsimd.tensor_scalar(
                        out=X[:, mn, :], in0=c_t[:, i, :],
                        scalar1=float(B1[i, m, n]), scalar2=float(B0[i, m, n]),
                        op0=MULT, op1=ADD)
                # Y[mn] = s * B2[mn]  (one broadcast TT over all 9 comps)
                s_b = s_t[:, i, :].unsqueeze(1).broadcast_to([_P, 9, U])
                b2_b = b2c[:, i, :].unsqueeze(2).broadcast_to([_P, 9, U])
                nc.gpsimd.tensor_tensor(out=Y[:], in0=s_b, in1=b2_b, op=MULT)
                if L == 1:
                    nc.gpsimd.tensor_tensor(
                        out=_ap(a, boff, [[4, 3], [1, 3], [144, U]]),
                        in0=X[:].rearrange("p (m n) j -> p m n j", m=3),
                        in1=Y[:].rearrange("p (m n) j -> p m n j", m=3), op=ADD)
                    continue
                Rl = scr.tile([_P, 9, U], f32, tag="Rl")
                nc.gpsimd.tensor_tensor(out=Rl[:], in0=X[:], in1=Y[:], op=ADD)

                # ---- t chain (VectorE STT) ----
                acc1 = scr.tile([_P, 3, U], f32, tag="acc1")
                acc2 = scr.tile([_P, 3, U], f32, tag="acc2")
                nc.vector.scalar_tensor_tensor(
                    out=acc1[:], in0=_ap(a, poff + 0, [[4, 3], [144, U]]),
                    scalar=float(TF[i, 0]),
                    in1=_ap(a, poff + 3, [[4, 3], [144, U]]), op0=MULT, op1=ADD)
                nc.vector.scalar_tensor_tensor(
                    out=acc2[:], in0=_ap(a, poff + 1, [[4, 3], [144, U]]),
                    scalar=float(TF[i, 1]), in1=acc1[:], op0=MULT, op1=ADD)
                nc.vector.scalar_tensor_tensor(
                    out=_ap(a, boff + 3, [[4, 3], [144, U]]),
                    in0=_ap(a, poff + 2, [[4, 3], [144, U]]),
                    scalar=float(TF[i, 2]), in1=acc2[:], op0=MULT, op1=ADD)

                # ---- R chain (VectorE): merged mult + 2 adds ----
                prod = scr.tile([_P, 3, 3, 3, U], f32, tag="prod")
                rp_b = _ap(a, poff, [[4, 3], [1, 3], [0, 3], [144, U]])
                rl_b = Rl[:].rearrange("p (k n) j -> p k n j", k=3).unsqueeze(1) \
                    .broadcast_to([_P, 3, 3, 3, U])
                nc.vector.tensor_tensor(out=prod[:], in0=rp_b, in1=rl_b, op=MULT)
                s9 = scr.tile([_P, 3, 3, U], f32, tag="s9")
                nc.vector.tensor_tensor(out=s9[:], in0=prod[:, :, 0], in1=prod[:, :, 1],
                                        op=ADD)
                nc.vector.tensor_tensor(
                    out=_ap(a, boff, [[4, 3], [1, 3], [144, U]]),
                    in0=s9[:], in1=prod[:, :, 2], op=ADD)

            nc.sync.dma_start(out=out_view[:, sl, :], in_=asm)


def trace_fk_v3(tc, out_ap, q_ap, b2c_ap, B0, B1, B2, TF, b_core, chunks,
                act_affine=5, fp16_chain=False, y_on="pool", add_on="pool",
                init_on="pool"):
    """v3: uneven chunks (small tail), link-1 on DVE, double-buffered Rl
    pipeline, affine split between ScalarE (act_affine comps) and GPSIMD.
    fp16_chain: chain state/products in fp16 (DVE 2x), asm writes stay fp32."""
    import concourse.bass as bass
    import concourse.mybir as mybir
    from contextlib import ExitStack

    nc = tc.nc
    f32 = mybir.dt.float32
    f16 = mybir.dt.float16
    cdt = f16 if fp16_chain else f32
    MULT = mybir.AluOpType.mult
    ADD = mybir.AluOpType.add
    COPY = mybir.ActivationFunctionType.Copy
    SIN = mybir.ActivationFunctionType.Sin

    ept = b_core // _P
    assert sum(chunks) == ept
    Umax = max(chunks)

    q_view = q_ap.rearrange("(p e) l -> p e l", p=_P)
    out_view = out_ap.rearrange("(p e) n r c -> p e (n r c)", p=_P)

    with ExitStack() as ctx:
        persist = ctx.enter_context(tc.tile_pool(name="persist", bufs=1))
        scr2 = ctx.enter_context(tc.tile_pool(name="scr2", bufs=2))
        scr1 = ctx.enter_context(tc.tile_pool(name="scr1", bufs=1))

        q_sb = persist.tile([_P, ept, 8], f32)
        nc.sync.dma_start(out=q_sb, in_=q_view)
        b2c = persist.tile([_P, _NL, 9], cdt)
        b2_bcast = bass.AP(tensor=b2c_ap.tensor, offset=b2c_ap.offset,
                           ap=[[0, _P], [9, _NL], [1, 9]])
        if fp16_chain:
            nc.gpsimd.dma_start(out=b2c, in_=b2_bcast)   # SWDGE casts f32->f16
        else:
            nc.sync.dma_start(out=b2c, in_=b2_bcast)

        asm_tiles = []
        for bi in range(2):
            asm = persist.tile([_P, Umax, 144], f32, tag=f"asm{bi}")
            a = asm[:]
            eng_i = nc.gpsimd if init_on == "pool" else nc.vector
            nc.vector.memset(_ap(a, 0, [[144, Umax], [1, 16]]), 0.0)
            nc.vector.memset(_ap(a, 0, [[144, Umax], [5, 4]]), 1.0)
            eng_i.memset(_ap(a, 16 + 12, [[144, Umax], [16, _NL], [1, 3]]), 0.0)
            eng_i.memset(_ap(a, 16 + 15, [[144, Umax], [16, _NL]]), 1.0)
            for m in range(3):
                eng_i.memset(_ap(a, 16 + 4 * m + 3, [[144, Umax]]),
                             float(TF[0, m]))
            asm_tiles.append(asm)

        base = 0
        for c, U in enumerate(chunks):
            asm = asm_tiles[c % 2]
            a = asm[:]
            sl = slice(base, base + U)
            base += U

            # ---- angles ----
            qs_t = scr2.tile([_P, _NL, U], f32, tag="qs")
            qc_t = scr2.tile([_P, _NL, U], f32, tag="qc")
            q_chunk = q_sb[:, sl, :].rearrange("p j l -> p l j")
            nc.vector.add_range_wrap(out=qs_t[:], in_=q_chunk, shift=0.0,
                                     bound=_PI, period=2 * _PI)
            s_t = scr2.tile([_P, _NL, U], cdt, tag="s")
            c_t = scr2.tile([_P, _NL, U], cdt, tag="c")
            if _COS_HALF:
                # cos(q) = 1 - 2 sin^2(q/2); q/2 of wrapped q is in [-pi/2,pi/2]
                # so no second range-wrap is needed. Square on ACT, affine on
                # GPSIMD (both have slack; saves one DVE pass per chunk).
                SQ = mybir.ActivationFunctionType.Square
                nc.scalar.activation(out=s_t[:, 0, :], in_=qs_t[:, 0, :], func=SIN)
                nc.scalar.activation(out=qc_t[:, 0, :], in_=qs_t[:, 0, :],
                                     func=SIN, scale=0.5)
                nc.scalar.activation(out=qc_t[:, 0, :], in_=qc_t[:, 0, :], func=SQ)
                nc.gpsimd.tensor_scalar(out=c_t[:, 0, :], in0=qc_t[:, 0, :],
                                        scalar1=-2.0, scalar2=1.0,
                                        op0=MULT, op1=ADD)
                nc.scalar.activation(out=s_t[:, 1:, :], in_=qs_t[:, 1:, :], func=SIN)
                nc.scalar.activation(out=qc_t[:, 1:, :], in_=qs_t[:, 1:, :],
                                     func=SIN, scale=0.5)
                nc.scalar.activation(out=qc_t[:, 1:, :], in_=qc_t[:, 1:, :], func=SQ)
                nc.gpsimd.tensor_scalar(out=c_t[:, 1:, :], in0=qc_t[:, 1:, :],
                                        scalar1=-2.0, scalar2=1.0,
                                        op0=MULT, op1=ADD)
            else:
                nc.vector.add_range_wrap(out=qc_t[:], in_=q_chunk, shift=_PI / 2,
                                         bound=_PI, period=2 * _PI)
                # link-1 trig first so the pipeline starts immediately
                nc.scalar.activation(out=s_t[:, 0, :], in_=qs_t[:, 0, :], func=SIN)
                nc.scalar.activation(out=c_t[:, 0, :], in_=qc_t[:, 0, :], func=SIN)
                nc.scalar.activation(out=s_t[:, 1:, :], in_=qs_t[:, 1:, :], func=SIN)
                nc.scalar.activation(out=c_t[:, 1:, :], in_=qc_t[:, 1:, :], func=SIN)

            pending_copy = None
            for L in range(1, _NL + 1):
                i = L - 1
                boff = L * 16
                poff = (L - 1) * 16

                if L == 1:
                    # fully on DVE (fills the chunk-start bubble):
                    # asm_R1[mn] = (s*B2 + (c*B1 + B0))
                    X1 = scr2.tile([_P, 9, U], cdt, tag="X1")
                    for mn in range(9):
                        m, n = divmod(mn, 3)
                        nc.vector.tensor_scalar(
                            out=X1[:, mn, :], in0=c_t[:, 0, :],
                            scalar1=float(B1[0, m, n]), scalar2=float(B0[0, m, n]),
                            op0=MULT, op1=ADD)
                    for mn in range(9):
                        m, n = divmod(mn, 3)
                        nc.vector.scalar_tensor_tensor(
                            out=_ap(a, boff + 4 * (mn // 3) + mn % 3, [[144, U]]),
                            in0=s_t[:, 0, :], scalar=float(B2[0, mn // 3, mn % 3]),
                            in1=X1[:, mn, :], op0=MULT, op1=ADD)
                    if fp16_chain:
                        R16p = scr2.tile([_P, 9, U], f16, tag="R16a")
                        nc.scalar.copy(
                            out=R16p[:],
                            in_=_ap(a, boff, [[4, 3], [1, 3], [144, U]]))
                    continue

                # ---- Rl (split: ScalarE affine for some comps, GPSIMD rest) ----
                X = scr2.tile([_P, 9, U], cdt, tag="X")
                Y = scr2.tile([_P, 9, U], cdt, tag="Y")
                if y_on == "aff":
                    # Rl = (c*B1 + B0/2) + (s*B2 + B0/2): 18 per-comp affines,
                    # first act_affine of the 18 on ScalarE, rest on GPSIMD.
                    ops = []
                    for mn in range(9):
                        m, n = divmod(mn, 3)
                        ops.append((X, mn, c_t, float(B1[i, m, n]),
                                    float(B0[i, m, n]) / 2))
                        ops.append((Y, mn, s_t, float(B2[i, m, n]),
                                    float(B0[i, m, n]) / 2))
                    for idx, (dst, mn, src, sc, bi_) in enumerate(ops):
                        if idx < act_affine:
                            nc.scalar.activation(
                                out=dst[:, mn, :], in_=src[:, i, :], func=COPY,
                                scale=sc, bias=bi_)
                        else:
                            nc.gpsimd.tensor_scalar(
                                out=dst[:, mn, :], in0=src[:, i, :],
                                scalar1=sc, scalar2=bi_, op0=MULT, op1=ADD)
                else:
                    for mn in range(9):
                        m, n = divmod(mn, 3)
                        if mn < act_affine:
                            nc.scalar.activation(
                                out=X[:, mn, :], in_=c_t[:, i, :], func=COPY,
                                scale=float(B1[i, m, n]), bias=float(B0[i, m, n]))
                        else:
                            nc.gpsimd.tensor_scalar(
                                out=X[:, mn, :], in0=c_t[:, i, :],
                                scalar1=float(B1[i, m, n]), scalar2=float(B0[i, m, n]),
                                op0=MULT, op1=ADD)
                    if y_on == "dve_ts":
                        for mn in range(9):
                            m, n = divmod(mn, 3)
                            nc.vector.tensor_scalar_mul(
                                out=Y[:, mn, :], in0=s_t[:, i, :],
                                scalar1=float(B2[i, m, n]))
                    else:
                        s_b = s_t[:, i, :].unsqueeze(1).broadcast_to([_P, 9, U])
                        b2_b = b2c[:, i, :].unsqueeze(2).broadcast_to([_P, 9, U])
                        nc.gpsimd.tensor_tensor(out=Y[:], in0=s_b, in1=b2_b, op=MULT)
                Rl = scr2.tile([_P, 9, U], cdt, tag="Rl")
                if add_on == "dve":
                    eng_a = nc.vector
                elif add_on == "split":
                    eng_a = nc.vector if L % 2 else nc.gpsimd
                else:
                    eng_a = nc.gpsimd
                eng_a.tensor_tensor(out=Rl[:], in0=X[:], in1=Y[:], op=ADD)
                if pending_copy is not None:
                    pending_copy()
                    pending_copy = None

                # ---- previous-R access ----
                if fp16_chain:
                    Rp_tile = R16p
                    def rp_k(k, _t=Rp_tile):
                        return _t[:].rearrange("p (m n) j -> p m n j", m=3) \
                            [:, :, k].unsqueeze(2).broadcast_to([_P, 3, 3, U])
                else:
                    def rp_k(k, _a=a, _poff=poff):
                        return _ap(_a, _poff + k, [[4, 3], [0, 3], [144, U]])

                # ---- t chain (fp32; DVE STT, or TS+TT pairs on GPSIMD) ----
                acc1 = scr1.tile([_P, 3, U], f32, tag="acc1")
                acc2 = scr1.tile([_P, 3, U], f32, tag="acc2")
                if fp16_chain:
                    def rp_col(k, _t=Rp_tile):
                        return _t[:].rearrange("p (m n) j -> p m n j", m=3)[:, :, k]
                else:
                    def rp_col(k, _a=a, _poff=poff):
                        return _ap(_a, _poff + k, [[4, 3], [144, U]])
                t_pool = (_T_POOL == "all") or (_T_POOL == "even" and L % 2 == 0)
                if t_pool:
                    u = scr1.tile([_P, 3, U], f32, tag="tu")
                    nc.gpsimd.tensor_scalar(out=u[:], in0=rp_col(0),
                                            scalar1=float(TF[i, 0]), scalar2=None,
                                            op0=MULT)
                    nc.gpsimd.tensor_tensor(
                        out=acc1[:], in0=u[:],
                        in1=_ap(a, poff + 3, [[4, 3], [144, U]]), op=ADD)
                    nc.gpsimd.tensor_scalar(out=u[:], in0=rp_col(1),
                                            scalar1=float(TF[i, 1]), scalar2=None,
                                            op0=MULT)
                    nc.gpsimd.tensor_tensor(out=acc2[:], in0=u[:], in1=acc1[:],
                                            op=ADD)
                    nc.gpsimd.tensor_scalar(out=u[:], in0=rp_col(2),
                                            scalar1=float(TF[i, 2]), scalar2=None,
                                            op0=MULT)
                    nc.gpsimd.tensor_tensor(
                        out=_ap(a, boff + 3, [[4, 3], [144, U]]),
                        in0=u[:], in1=acc2[:], op=ADD)
                else:
                    nc.vector.scalar_tensor_tensor(
                        out=acc1[:], in0=rp_col(0), scalar=float(TF[i, 0]),
                        in1=_ap(a, poff + 3, [[4, 3], [144, U]]), op0=MULT, op1=ADD)
                    nc.vector.scalar_tensor_tensor(
                        out=acc2[:], in0=rp_col(1), scalar=float(TF[i, 1]),
                        in1=acc1[:], op0=MULT, op1=ADD)
                    nc.vector.scalar_tensor_tensor(
                        out=_ap(a, boff + 3, [[4, 3], [144, U]]),
                        in0=rp_col(2), scalar=float(TF[i, 2]), in1=acc2[:],
                        op0=MULT, op1=ADD)

                # ---- R chain (DVE) ----
                def rl_k(k):
                    return Rl[:, 3 * k:3 * k + 3, :].unsqueeze(1).broadcast_to(
                        [_P, 3, 3, U])

                # per-k products (ISA limit: <=3 free dims per AP)
                prod = scr1.tile([_P, 3, 3, 3, U], cdt, tag="prod")
                for k in range(3):
                    nc.vector.tensor_tensor(out=prod[:, :, k], in0=rp_k(k),
                                            in1=rl_k(k), op=MULT)
                s9b = scr1.tile([_P, 3, 3, U], cdt, tag="s9b")
                eng_a1 = nc.gpsimd if (_A1_POOL and L % 2 == 0) else nc.vector
                eng_a1.tensor_tensor(out=s9b[:], in0=prod[:, :, 0],
                                     in1=prod[:, :, 1], op=ADD)
                if fp16_chain:
                    R16n = scr2.tile([_P, 9, U], f16, tag="R16b" if L % 2 else "R16a")
                    nc.vector.tensor_tensor(
                        out=R16n[:].rearrange("p (m n) j -> p m n j", m=3),
                        in0=s9b[:], in1=prod[:, :, 2], op=ADD)

                    def _copy(_R=R16n, _boff=boff):
                        nc.scalar.copy(
                            out=_ap(a, _boff, [[4, 3], [1, 3], [144, U]]),
                            in_=_R[:].rearrange("p (m n) j -> p m n j", m=3))
                    pending_copy = _copy
                    R16p = R16n
                else:
                    nc.vector.tensor_tensor(
                        out=_ap(a, boff, [[4, 3], [1, 3], [144, U]]),
                        in0=s9b[:], in1=prod[:, :, 2], op=ADD)

            if pending_copy is not None:
                pending_copy()
                pending_copy = None
            nc.sync.dma_start(out=out_view[:, sl, :], in_=asm[:, :U, :])


def trace_fk_v4(tc, out_aps, q_ap, B0, B1, B2, TF, b_core,
                x_on="act", y_act_links=0, t_on="pool", cos_half=False,
                rl_pool_links=0, pool_cols=0, pool_scan=False,
                tu_act=False, x_lazy=False):
    """v4: all-fp16 compute, compact component-major tiles [P, comp, U]
    (elem innermost, unit stride => DVE 2x TT / 4x TS modes), only the 93
    non-constant output components DMA'd to HBM as fp16 [8, 12, b_core]
    (host reassembles [B,9,4,4] fp32 and fills constants).

    Engine split: ACT trig + X=c*B1+B0 affines; DVE Y=s*B2 (4x TS),
    Rl=X+Y (2x TT), 3x3 chain products+adds (2x TT); Pool t-chain (STT,
    off critical path) and optionally some Rl adds."""
    import concourse.bass as bass
    import concourse.mybir as mybir
    from contextlib import ExitStack

    nc = tc.nc
    f32 = mybir.dt.float32
    f16 = mybir.dt.float16
    MULT = mybir.AluOpType.mult
    ADD = mybir.AluOpType.add
    BYPASS = mybir.AluOpType.bypass
    COPY = mybir.ActivationFunctionType.Copy
    SIN = mybir.ActivationFunctionType.Sin
    SQ = mybir.ActivationFunctionType.Square

    ept = b_core // _P
    U = ept

    def _t_pool(t_on_):
        return int(t_on_) if str(t_on_).isdigit() else (8 if t_on_ == "pool" else 0)

    q_view = q_ap.rearrange("(p e) l -> p e l", p=_P)   # [P, ept, 8]

    def dram_links(kind, a, b, Wk):
        # per-lane out tensor [P, NL, 12, Wk]: per partition the whole
        # [NL,12,Wk] block is contiguous -> 128 fat descriptors per DMA
        oap = out_aps[0] if kind == "dve" else out_aps[1]
        nl = b - a + 1
        return bass.AP(tensor=oap.tensor,
                       offset=oap.offset + (a - 1) * 12 * Wk,
                       ap=[[_NL * 12 * Wk, _P], [12 * Wk, nl],
                           [Wk, 12], [1, Wk]])

    with ExitStack() as ctx:
        persist = ctx.enter_context(tc.tile_pool(name="persist", bufs=1))
        scr2 = ctx.enter_context(tc.tile_pool(name="scr2", bufs=2))
        scr1 = ctx.enter_context(tc.tile_pool(name="scr1", bufs=1))

        pool_cols = pool_cols * U // 256   # knob calibrated at U=256
        Dd = U - pool_cols
        # ramp pieces: Pool's columns [Dd, U) first so its sub-chain starts
        # as early as possible, then the DVE columns
        pieces = ([(Dd, U - Dd), (0, Dd)] if pool_cols else [(0, U)])

        q_sb = persist.tile([_P, ept, 8], f32)
        for c0, W in pieces:
            nc.sync.dma_start(out=q_sb[:, c0:c0 + W, :],
                              in_=q_view[:, c0:c0 + W, :])

        qlv = q_sb[:].rearrange("p e l -> p l e")        # [P, 8, ept] strided
        qs_t = persist.tile([_P, _NL, U], f32, tag="qs")
        s_t = persist.tile([_P, _NL, U], f16, tag="s")
        c_t = persist.tile([_P, _NL, U], f16, tag="c")
        qc_t = persist.tile([_P, _NL, U], f32, tag="qc")
        u_t = persist.tile([_P, _NL, U], f16, tag="u")
        for c0, W in pieces:
            sl = slice(c0, c0 + W)
            nc.vector.add_range_wrap(out=qs_t[:, :, sl], in_=qlv[:, :, sl],
                                     shift=0.0, bound=_PI, period=2 * _PI)
            nc.scalar.activation(out=s_t[:, :, sl], in_=qs_t[:, :, sl],
                                 func=SIN)
            if cos_half:
                # cos(q) = 1 - 2 sin^2(q/2); wrapped q/2 in [-pi/2, pi/2]
                nc.scalar.activation(out=u_t[:, :, sl], in_=qs_t[:, :, sl],
                                     func=SIN, scale=0.5)
                nc.scalar.activation(out=u_t[:, :, sl], in_=u_t[:, :, sl],
                                     func=SQ)
                nc.vector.tensor_scalar(out=c_t[:, :, sl], in0=u_t[:, :, sl],
                                        scalar1=-2.0, scalar2=1.0,
                                        op0=MULT, op1=ADD)
            else:
                nc.vector.add_range_wrap(out=qc_t[:, :, sl], in_=qlv[:, :, sl],
                                         shift=_PI / 2, bound=_PI,
                                         period=2 * _PI)
                nc.scalar.activation(out=c_t[:, :, sl], in_=qc_t[:, :, sl],
                                     func=SIN)

        # X[i] = c_i*B1[i] + B0[i]  (all links, ahead of the chain)
        Xall = persist.tile([_P, _NL, 9, U], f16, tag="X")

        def emit_x(i, c0, W):
            for mn in range(9):
                m, n = divmod(mn, 3)
                if x_on == "act":
                    nc.scalar.activation(
                        out=Xall[:, i, mn, c0:c0 + W],
                        in_=c_t[:, i, c0:c0 + W], func=COPY,
                        scale=float(B1[i, m, n]), bias=float(B0[i, m, n]))
                else:
                    nc.vector.tensor_scalar(
                        out=Xall[:, i, mn, c0:c0 + W],
                        in0=c_t[:, i, c0:c0 + W],
                        scalar1=float(B1[i, m, n]), scalar2=float(B0[i, m, n]),
                        op0=MULT, op1=ADD)

        for c0, W in pieces:
            emit_x(0, c0, W)
        if not x_lazy:
            for i in range(1, _NL):
                emit_x(i, 0, U)

        Yall = persist.tile([_P, _NL, 9, U], f16, tag="Yall")

        # element-column split: DVE gets [0, D), Pool replays the same
        # algorithm on [D, U) (its own independent sub-chain). Per-lane RT
        # tiles + per-lane HBM tensors: fully disjoint memory, so the two
        # lanes can NEVER false-depend on each other's byte ranges.
        D = Dd
        subs = [("dve", 0, D)]
        if pool_cols:
            subs = [("pool", D, pool_cols)] + subs
        RTd = persist.tile([_P, _NL, 12, D], f16, tag="RTd")
        rt_tiles = {"dve": RTd}
        if pool_cols:
            RTp = persist.tile([_P, _NL, 12, pool_cols], f16, tag="RTp")
            rt_tiles["pool"] = RTp
        lane_w = dict((k, w) for k, _, w in subs)

        def rt_ap(kind, L, comp0, dims):
            rtb = rt_tiles[kind][:]
            Wk = lane_w[kind]
            off = ((L - 1) * 12 + comp0) * Wk
            return bass.AP(tensor=rtb.tensor, offset=rtb.offset + off,
                           ap=[list(rtb.ap[0])] + [list(d) for d in dims])

        # t_1 = tf_1 (constant)
        for kind, c0, W in subs:
            for m in range(3):
                nc.gpsimd.memset(rt_ap(kind, 1, 9 + m, [[1, W]]),
                                 float(TF[0, m]))

        # --- software-pipelined emission: Y/Rl built 2 links ahead so the
        # chain's dependent ops always have independent work in front of
        # them (hides the ~0.5us per-op dependency stall seen on HW) ---
        scr3 = ctx.enter_context(tc.tile_pool(name="scr3", bufs=3))
        rl_tiles = {}

        def emit_y(L):
            i = L - 1
            for yc0, yW in (pieces if L == 1 else [(0, U)]):
                for mn in range(9):
                    m, n = divmod(mn, 3)
                    if i < y_act_links:
                        nc.scalar.activation(
                            out=Yall[:, i, mn, yc0:yc0 + yW],
                            in_=s_t[:, i, yc0:yc0 + yW],
                            func=COPY, scale=float(B2[i, m, n]))
                    else:
                        nc.vector.tensor_scalar_mul(
                            out=Yall[:, i, mn, yc0:yc0 + yW],
                            in0=s_t[:, i, yc0:yc0 + yW],
                            scalar1=float(B2[i, m, n]))

        def emit_rl(L):
            i = L - 1
            for kind, c0, W in subs:
                e = nc.vector if kind == "dve" else nc.gpsimd
                Rl = scr3.tile([_P, 9, W], f16, tag=f"Rl{kind}")
                rl_tiles[(kind, L)] = Rl
                e.tensor_tensor(out=Rl[:], in0=Xall[:, i, :, c0:c0 + W],
                                in1=Yall[:, i, :, c0:c0 + W], op=ADD)

        emit_y(1)
        for kind, c0, W in subs:
            e = nc.vector if kind == "dve" else nc.gpsimd
            e.tensor_tensor(out=rt_ap(kind, 1, 0, [[W, 9], [1, W]]),
                            in0=Xall[:, 0, :, c0:c0 + W],
                            in1=Yall[:, 0, :, c0:c0 + W], op=ADD)
        emit_y(2)
        emit_rl(2)
        emit_y(3)
        emit_rl(3)

        for L in range(2, _NL + 1):
            i = L - 1
            for kind, c0, W in subs:
                eng = nc.vector if kind == "dve" else nc.gpsimd
                sl = slice(c0, c0 + W)
                Rl = rl_tiles.pop((kind, L))
                p = scr1.tile([_P, 27, W], f16, tag=f"p{kind}")
                s9 = scr1.tile([_P, 9, W], f16, tag=f"s9{kind}")
                u = scr1.tile([_P, 3, 3, W], f16, tag=f"tu{kind}")
                a1 = scr1.tile([_P, 3, W], f16, tag=f"a1{kind}")
                a2 = scr1.tile([_P, 3, W], f16, tag=f"a2{kind}")
                pb = p[:]
                rlb = Rl[:]
                # products first (Rl_L finished ~1 link ago; no stall)
                for k in range(3):
                    eng.tensor_tensor(
                        out=bass.AP(tensor=pb.tensor,
                                    offset=pb.offset + 9 * k * W,
                                    ap=[list(pb.ap[0]), [3 * W, 3], [W, 3],
                                        [1, W]]),
                        in0=rt_ap(kind, L - 1, k,
                                  [[3 * W, 3], [0, 3], [1, W]]),
                        in1=bass.AP(tensor=rlb.tensor,
                                    offset=rlb.offset + 3 * k * W,
                                    ap=[list(rlb.ap[0]), [0, 3], [W, 3],
                                        [1, W]]),
                        op=MULT)
                # independent t-chain scalar products fill the p->s9 gap
                for n in range(3):
                    if tu_act:
                        nc.scalar.activation(
                            out=u[:, n],
                            in_=rt_ap(kind, L - 1, n, [[3 * W, 3], [1, W]]),
                            func=COPY, scale=float(TF[i, n]))
                    else:
                        eng.tensor_scalar_mul(
                            out=u[:, n],
                            in0=rt_ap(kind, L - 1, n, [[3 * W, 3], [1, W]]),
                            scalar1=float(TF[i, n]))
                eng.tensor_tensor(out=s9[:], in0=p[:, 0:9], in1=p[:, 9:18],
                                  op=ADD)
                eng.tensor_tensor(out=a1[:], in0=u[:, 0], in1=u[:, 1], op=ADD)
                eng.tensor_tensor(out=a2[:], in0=u[:, 2],
                                  in1=rt_ap(kind, L - 1, 9, [[W, 3], [1, W]]),
                                  op=ADD)
                eng.tensor_tensor(out=rt_ap(kind, L, 0, [[W, 9], [1, W]]),
                                  in0=s9[:], in1=p[:, 18:27], op=ADD)
                eng.tensor_tensor(out=rt_ap(kind, L, 9, [[W, 3], [1, W]]),
                                  in0=a1[:], in1=a2[:], op=ADD)
            if L in (3, 6, _NL):
                a_, b_ = {3: (1, 3), 6: (4, 6), _NL: (7, 8)}[L]
                for kind, c0, W in subs:
                    nc.sync.dma_start(
                        out=dram_links(kind, a_, b_, W),
                        in_=rt_tiles[kind][:, a_ - 1:b_])
            if L + 2 <= _NL:
                emit_y(L + 2)
                emit_rl(L + 2)
            if x_lazy and L < _NL:
                emit_x(L, 0, U)


def assemble_v4(core_outs, TF, core_outs_p=None):
    """core_outs: per-core [P, NL, 12, D] fp16 (DVE lane); core_outs_p:
    per-core [P, NL, 12, pc] (Pool lane cols) -> [B, 9, 4, 4] f32."""
    first = np.asarray(core_outs[0])
    D = first.size // (_NL * _P * 12)
    pc = 0
    if core_outs_p is not None:
        pc = np.asarray(core_outs_p[0]).size // (_NL * _P * 12)
    ept = D + pc
    b_core = _P * ept
    B = b_core * len(core_outs)
    full = np.zeros((B, 9, 4, 4), dtype=np.float32)
    full[:, :, 3, 3] = 1.0
    full[:, 0, 0, 0] = 1.0
    full[:, 0, 1, 1] = 1.0
    full[:, 0, 2, 2] = 1.0
    for ci in range(len(core_outs)):
        a = np.asarray(core_outs[ci]).reshape(_P, _NL, 12, D)
        if pc:
            ap_ = np.asarray(core_outs_p[ci]).reshape(_P, _NL, 12, pc)
            a = np.concatenate([a, ap_], axis=-1)
        a = a.astype(np.float32).transpose(0, 3, 1, 2).reshape(
            b_core, _NL, 12)
        b0 = ci * b_core
        full[b0:b0 + b_core, 1:, :3, :3] = a[:, :, :9].reshape(b_core, _NL, 3, 3)
        full[b0:b0 + b_core, 1:, :3, 3] = a[:, :, 9:12]
    return full


import os as _os
_VARIANT = _os.environ.get("KERNEL_VARIANT", "v4")


def _build_program(B0, B1, B2, TF, b_core, chunk, variant=None, reps=1):
    import concourse.bacc as bacc
    import concourse.mybir as mybir
    import concourse.tile as tile

    variant = variant or _VARIANT
    nc = bacc.Bacc("TRN2", target_bir_lowering=False, debug=False)
    q_d = nc.dram_tensor("q", [b_core, _NL], mybir.dt.float32, kind="ExternalInput")
    if variant == "v4":
        ept = b_core // _P
        pc = _V4_POOL_COLS * ept // 256
        Dd = ept - pc
        out_d = nc.dram_tensor("out", [_P, _NL, 12, Dd], mybir.dt.float16,
                               kind="ExternalOutput")
        outp_d = nc.dram_tensor("outp", [_P, _NL, 12, max(pc, 1)],
                                mybir.dt.float16, kind="ExternalOutput")
        with tile.TileContext(nc) as tc:
            def _body():
                trace_fk_v4(tc, [out_d.ap(), outp_d.ap()], q_d.ap(),
                            B0, B1, B2, TF, b_core,
                            x_on=_V4_X_ON, y_act_links=_V4_Y_ACT,
                            t_on=_V4_T_ON, cos_half=_V4_COS_HALF,
                            rl_pool_links=_V4_RL_POOL, pool_cols=_V4_POOL_COLS,
                            pool_scan=_V4_POOL_SCAN, tu_act=_V4_TU_ACT,
                            x_lazy=_V4_X_LAZY)
            if reps == 1:
                _body()
            else:
                with tc.For_i(0, reps):
                    _body()
        nc.compile()
        return nc
    out_d = nc.dram_tensor("out", [b_core, 9, 4, 4], mybir.dt.float32,
                           kind="ExternalOutput")
    if variant == "v1":
        with tile.TileContext(nc) as tc:
            trace_fk(tc, out_d.ap(), q_d.ap(), B0, B1, B2, TF, b_core, chunk)
    elif variant == "v2":
        b2c_d = nc.dram_tensor("b2c", [_NL * 9], mybir.dt.float32,
                               kind="ExternalInput")
        with tile.TileContext(nc) as tc:
            trace_fk_v2(tc, out_d.ap(), q_d.ap(), b2c_d.ap(), B0, B1, B2, TF,
                        b_core, chunk)
    else:
        b2c_d = nc.dram_tensor("b2c", [_NL * 9], mybir.dt.float32,
                               kind="ExternalInput")
        ept = b_core // _P
        chunks = _mk_chunks(ept)
        fp16 = (variant == "v3fp16")
        aa = _ACT_AFFINE if _ACT_AFFINE is not None else (10 if fp16 else 9)
        yo = _Y_ON if _Y_ON is not None else ("aff" if fp16 else "pool")
        ao = _ADD_ON if _ADD_ON is not None else ("split" if fp16 else "pool")
        with tile.TileContext(nc) as tc:
            def _body3():
                trace_fk_v3(tc, out_d.ap(), q_d.ap(), b2c_d.ap(), B0, B1, B2,
                            TF, b_core, chunks, act_affine=aa, fp16_chain=fp16,
                            y_on=yo, add_on=ao, init_on=_INIT_ON)
            if reps == 1:
                _body3()
            else:
                with tc.For_i(0, reps):
                    _body3()
    nc.compile()
    return nc


_ACT_AFFINE = None
_Y_ON = None
_ADD_ON = None
_INIT_ON = "pool"
_A1_POOL = True
_COS_HALF = False
_T_POOL = "none"
_CHUNKS_FRACS = (0.40625, 0.40625, 0.1875)   # 104/104/48 at ept=256

# v4 knobs
_V4_X_ON = "dve"      # "act" | "dve"
_V4_Y_ACT = 8         # links whose Y=s*B2 runs on ACT instead of DVE
_V4_T_ON = "dve"      # "dve" | "pool" | int (first N links' t on Pool)
_V4_COS_HALF = False  # cos via 1-2sin^2(q/2) (drops 2nd range wrap)
_V4_RL_POOL = 0       # links whose Rl=X+Y add runs on Pool
_V4_POOL_COLS = 48    # element columns handled by a Pool-side sub-chain
_V4_POOL_SCAN = False  # Pool elementwise via tensor_tensor_scan(bypass)
_V4_TU_ACT = True      # t-chain scalar products on ACT (both lanes)
_V4_X_LAZY = False     # emit X_{L+1} inside the link loop (ACT ordering)


def _mk_chunks(ept):
    if ept <= 32:
        return [ept]
    cs = [max(8, int(ept * f) // 8 * 8) for f in _CHUNKS_FRACS[:-1]]
    cs.append(ept - sum(cs))
    assert all(x > 0 for x in cs)
    return cs


def kernel(q, fixed_rot, fixed_trans, joint_axis):
    from concourse import bass_utils

    q = np.asarray(q, dtype=np.float32)
    B = q.shape[0]
    b_core = B // _N_CORES
    B0, B1, B2, TF = _fk_constants(np.asarray(fixed_rot), np.asarray(fixed_trans),
                                   np.asarray(joint_axis))
    nc = _build_program(B0, B1, B2, TF, b_core, _U)
    in_maps = [{"q": np.ascontiguousarray(q[i * b_core:(i + 1) * b_core])}
               for i in range(_N_CORES)]
    if _VARIANT not in ("v1", "v4"):
        b2c = np.ascontiguousarray(B2.reshape(-1).astype(np.float32))
        for m in in_maps:
            m["b2c"] = b2c
    res = bass_utils.run_bass_kernel_spmd(nc, in_maps, core_ids=list(range(_N_CORES)))
    if _VARIANT == "v4":
        return assemble_v4([res.results[i]["out"] for i in range(_N_CORES)],
                           TF,
                           [res.results[i]["outp"] for i in range(_N_CORES)]
                           if _V4_POOL_COLS else None)
    out = np.concatenate([res.results[i]["out"] for i in range(_N_CORES)], axis=0)
    return out.astype(np.float32)



# revision 49
# speedup vs baseline: 1.1306x; 1.1306x over previous
"""Trainium2 Bass kernel for batched 8-link serial-chain forward kinematics.

Problem: for each batch element b with joint angles q[b, 0:8]:
    T_0 = I4
    T_i = T_{i-1} @ [[Rf_i, tf_i],[0,1]] @ [[Rj(q_i), 0],[0,1]]
    output[b] = stack(T_0 .. T_8)  -> [B, 9, 4, 4] float32

Current default variant "v4" (same-day HW A/B: 115.5us vs v3's 252.7us,
i.e. ~2.2x faster; the absolute number varies ~1.8x with the axon device
state, so compare variants same-day same-method only). Knobs x=dve,
y_act=8, tu_act=1, pool_cols=48: moving the 114 small TS instructions
(Y=s*B2 and the t-chain u_n=Rp*tf products) from DVE to ACT bought 19%
on HW (~0.25us/instr net, far more than the cost model predicts —
DVE per-instruction overhead on silicon is the dominant unmodeled cost;
prefer fewer/bigger DVE ops and park small ops on ACT):
  - all-fp16 compute in compact component-major tiles [P, comp, elem]
    (elem innermost unit-stride -> DVE 2x TT / 4x TS perf modes; verified
    on HW: indep fp16 TT [9,256] = 1301ns, chained +490ns RAW stall,
    fp32 TT = 2525ns);
  - only the 93 non-constant output components are written to HBM as
    fp16 [P, NL, 12, ept] (3 batched fat DMAs, 24KB contiguous per
    partition); the host fills I4/bottom-row/t_1 constants and casts;
  - ACT: trig (Sin table is ONLY valid on [-pi,pi]: 1e-7 err inside,
    garbage beyond |x|>4 -> both range-wraps are mandatory) + X=c*B1+B0;
  - DVE: wraps, Y=s*B2 (TS), Rl=X+Y, 3x3 chain products+adds, t-chain;
  - Pool: an independent sub-chain replaying the same algorithm on the
    last 48 element-columns (element-split data parallelism);
  - emission is software-pipelined: Y/Rl built 2 links ahead, t-chain
    scalar products interleaved between dependent chain ops;
  - timing methodology: per-dispatch tunnel latency is ~70-100ms(!), so
    test.py benches wall(For_i x199 in-NEFF) vs wall(1) — the only
    reliable method here (old slope-minus-trivial method gives garbage).

Older variant "v3" (fp32 element-major, recorded 142us):
  - Each core handles B/8 = 32768 elements: 128 partitions x 256
    elements/partition, processed in column-chunks (large..small so the
    final output DMA tail is short), double-buffered assembly tiles.
  - Rodrigues rotation as a linear form: Rl_i = B0_i + cos(q_i)*B1_i +
    sin(q_i)*B2_i with per-link constant 3x3 matrices (precomputed on host,
    baked into the instruction stream as immediates).
  - cos/sin on ScalarE Sin spline (inputs range-wrapped into [-pi,pi] by the
    custom add_range_wrap DVE op; cos = sin(q + pi/2)).
  - Rl construction off the critical engine: affine c*B1+B0 on ScalarE
    (activation Copy with scale/bias immediates), s*B2 via a broadcast
    tensor_tensor against a replicated B2 constants tile plus the final add
    on GPSIMD.
  - 3x3 chain products on VectorE: per-k merged multi-dim-AP tensor_tensor
    (broadcast dims; <=3 free dims per AP, a hardware ISA limit), reading
    R_{L-1} strided straight out of the assembly tile.
  - Translation chain: 3 fused scalar_tensor_tensor ops on VectorE.
  - Link 1 runs entirely on VectorE (tensor_scalar + scalar_tensor_tensor)
    to fill the chunk-start pipeline bubble.
  - Results are written directly (strided APs) into an element-major
    assembly tile [128, U, 144]; constant positions (T_0 = I4, bottom rows,
    t_1 = tf_1) are memset once per buffer and never rewritten. One big
    contiguous DMA per chunk writes U*144 floats per partition to HBM.
  - A "v3fp16" variant (chain state in fp16 for DVE 2x mode) exists but is
    not the default: it reaches ~3e-3 max abs error vs the fp32 reference.
"""

import numpy as np

_P = 128          # SBUF partitions
_NL = 8           # moving links
_B_FULL = 262144  # full batch
_N_CORES = 8
_U = 128          # elements per partition per chunk

_PI = float(np.pi)


def _fk_constants(fixed_rot, fixed_trans, joint_axis):
    """Per-link constants: Rl_i(q) = B0 + cos(q) B1 + sin(q) B2."""
    fr = np.asarray(fixed_rot, dtype=np.float64)     # [8,3,3]
    ax = np.asarray(joint_axis, dtype=np.float64)    # [8,3]
    tf = np.asarray(fixed_trans, dtype=np.float64)   # [8,3]
    B0 = np.zeros((_NL, 3, 3)); B1 = np.zeros((_NL, 3, 3)); B2 = np.zeros((_NL, 3, 3))
    I3 = np.eye(3)
    for i in range(_NL):
        a = ax[i]
        K = np.array([[0, -a[2], a[1]], [a[2], 0, -a[0]], [-a[1], a[0], 0]])
        aa = np.outer(a, a)
        B0[i] = fr[i] @ aa
        B1[i] = fr[i] @ (I3 - aa)
        B2[i] = fr[i] @ K
    return (B0.astype(np.float32), B1.astype(np.float32), B2.astype(np.float32),
            tf.astype(np.float32))


def _ap(base, extra_off, dims):
    """Build a custom AP on the same tensor as `base` (an AP), keeping its
    partition dim. dims = list of [step, count] in elements."""
    import concourse.bass as bass
    return bass.AP(tensor=base.tensor, offset=base.offset + extra_off,
                   ap=[list(base.ap[0])] + [list(d) for d in dims])


def trace_fk(tc, out_ap, q_ap, B0, B1, B2, TF, b_core, chunk):
    """Trace the per-core FK program into TileContext `tc`.

    out_ap: DRAM [b_core, 9, 4, 4] f32;  q_ap: DRAM [b_core, 8] f32.
    """
    import concourse.mybir as mybir
    from contextlib import ExitStack

    nc = tc.nc
    f32 = mybir.dt.float32
    MULT = mybir.AluOpType.mult
    ADD = mybir.AluOpType.add
    COPY = mybir.ActivationFunctionType.Copy
    SIN = mybir.ActivationFunctionType.Sin

    ept = b_core // _P              # elements per partition (total)
    U = chunk
    nchunks = ept // U
    assert ept % U == 0

    q_view = q_ap.rearrange("(p e) l -> p e l", p=_P)            # [P, ept, 8]
    out_view = out_ap.rearrange("(p e) n r c -> p e (n r c)", p=_P)  # [P, ept, 144]

    with ExitStack() as ctx:
        persist = ctx.enter_context(tc.tile_pool(name="persist", bufs=1))
        scr = ctx.enter_context(tc.tile_pool(name="scr", bufs=1))
        work = scr

        # --- persistent tiles ---
        q_sb = persist.tile([_P, ept, 8], f32)
        nc.sync.dma_start(out=q_sb, in_=q_view)

        asm_tiles = []
        for bi in range(min(2, nchunks)):
            asm = persist.tile([_P, U, 144], f32, tag=f"asm{bi}")
            a = asm[:]
            # T_0 = eye(4): zero the 16-float block, then ones on the diag.
            nc.vector.memset(_ap(a, 0, [[144, U], [1, 16]]), 0.0)
            nc.vector.memset(_ap(a, 0, [[144, U], [5, 4]]), 1.0)
            # bottom rows of T_1..T_8: [0,0,0,1]
            nc.gpsimd.memset(_ap(a, 16 + 12, [[144, U], [16, _NL], [1, 3]]), 0.0)
            nc.gpsimd.memset(_ap(a, 16 + 15, [[144, U], [16, _NL]]), 1.0)
            # t_1 = tf_1 (constant: R_0 = I, t_0 = 0)
            for m in range(3):
                nc.gpsimd.memset(_ap(a, 16 + 4 * m + 3, [[144, U]]), float(TF[0, m]))
            asm_tiles.append(asm)

        for c in range(nchunks):
            asm = asm_tiles[c % 2]
            a = asm[:]
            sl = slice(c * U, (c + 1) * U)

            # ---- angles: wrap + trig, [P, 8, U] link-major ----
            qs_t = work.tile([_P, _NL, U], f32, tag="X")
            qc_t = work.tile([_P, _NL, U], f32, tag="Rl")
            q_in = q_view  # noqa
            q_chunk = q_sb[:, sl, :].rearrange("p j l -> p l j")
            nc.vector.add_range_wrap(out=qs_t[:], in_=q_chunk, shift=0.0,
                                     bound=_PI, period=2 * _PI)
            nc.vector.add_range_wrap(out=qc_t[:], in_=q_chunk, shift=_PI / 2,
                                     bound=_PI, period=2 * _PI)
            s_t = work.tile([_P, _NL, U], f32, tag="s")
            c_t = work.tile([_P, _NL, U], f32, tag="c")
            nc.scalar.activation(out=s_t[:], in_=qs_t[:], func=SIN)
            nc.scalar.activation(out=c_t[:], in_=qc_t[:], func=SIN)

            for L in range(1, _NL + 1):
                i = L - 1           # input link row
                boff = L * 16       # output T-block offset in asm
                poff = (L - 1) * 16  # previous T-block

                # ---- X = c*B1 + B0 (ScalarE), Rl = X + s*B2 (GPSIMD) ----
                X = scr.tile([_P, 9, U], f32, tag="X")
                for mn in range(9):
                    m, n = divmod(mn, 3)
                    nc.gpsimd.tensor_scalar(
                        out=X[:, mn, :], in0=c_t[:, i, :],
                        scalar1=float(B1[i, m, n]), scalar2=float(B0[i, m, n]),
                        op0=MULT, op1=ADD)
                if L == 1:
                    # R_1 = Rl_1 directly into asm (R_0 = I): strided out
                    for mn in range(9):
                        m, n = divmod(mn, 3)
                        nc.vector.scalar_tensor_tensor(
                            out=_ap(a, boff + 4 * m + n, [[144, U]]),
                            in0=s_t[:, i, :], scalar=float(B2[i, m, n]),
                            in1=X[:, mn, :], op0=MULT, op1=ADD)
                    continue

                Rl = scr.tile([_P, 9, U], f32, tag="Rl")
                for mn in range(9):
                    m, n = divmod(mn, 3)
                    nc.vector.scalar_tensor_tensor(
                        out=Rl[:, mn, :], in0=s_t[:, i, :], scalar=float(B2[i, m, n]),
                        in1=X[:, mn, :], op0=MULT, op1=ADD)

                # ---- t chain (GPSIMD): t_L = R_{L-1} tf_L + t_{L-1} ----
                # merged over m: in0 = R_{L-1}[:, k] col (strided from asm)
                acc1 = scr.tile([_P, 3, U], f32, tag="acc1")
                acc2 = scr.tile([_P, 3, U], f32, tag="acc2")
                nc.vector.scalar_tensor_tensor(
                    out=acc1[:], in0=_ap(a, poff + 0, [[4, 3], [144, U]]),
                    scalar=float(TF[i, 0]),
                    in1=_ap(a, poff + 3, [[4, 3], [144, U]]), op0=MULT, op1=ADD)
                nc.vector.scalar_tensor_tensor(
                    out=acc2[:], in0=_ap(a, poff + 1, [[4, 3], [144, U]]),
                    scalar=float(TF[i, 1]), in1=acc1[:], op0=MULT, op1=ADD)
                nc.vector.scalar_tensor_tensor(
                    out=_ap(a, boff + 3, [[4, 3], [144, U]]),
                    in0=_ap(a, poff + 2, [[4, 3], [144, U]]),
                    scalar=float(TF[i, 2]), in1=acc2[:], op0=MULT, op1=ADD)

                # ---- R chain (VectorE): R_L = R_{L-1} @ Rl ----
                # mk: s9[m,n] = Rp[m,k] * Rl[k,n]  (broadcast over n / m)
                s9a = scr.tile([_P, 3, 3, U], f32, tag="s9a")
                s9b = scr.tile([_P, 3, 3, U], f32, tag="s9b")
                s9c = scr.tile([_P, 3, 3, U], f32, tag="s9c")

                def rp_k(k):
                    return _ap(a, poff + k, [[4, 3], [0, 3], [144, U]])

                def rl_k(k):
                    return Rl[:, 3 * k:3 * k + 3, :].unsqueeze(1).broadcast_to(
                        [_P, 3, 3, U])

                nc.vector.tensor_tensor(out=s9a[:], in0=rp_k(0), in1=rl_k(0), op=MULT)
                nc.vector.tensor_tensor(out=s9b[:], in0=rp_k(1), in1=rl_k(1), op=MULT)
                nc.vector.tensor_tensor(out=s9b[:], in0=s9a[:], in1=s9b[:], op=ADD)
                nc.vector.tensor_tensor(out=s9c[:], in0=rp_k(2), in1=rl_k(2), op=MULT)
                nc.vector.tensor_tensor(
                    out=_ap(a, boff, [[4, 3], [1, 3], [144, U]]),
                    in0=s9b[:], in1=s9c[:], op=ADD)

            # ---- store chunk ----
            nc.sync.dma_start(out=out_view[:, sl, :], in_=asm)


def trace_fk_v2(tc, out_ap, q_ap, b2c_ap, B0, B1, B2, TF, b_core, chunk):
    """v2: Rl construction fully on GPSIMD (affine tensor_scalar + broadcast
    tensor_tensor with a B2 constants tile); chain mult merged into one
    tensor_tensor per link on VectorE."""
    import concourse.bass as bass
    import concourse.mybir as mybir
    from contextlib import ExitStack

    nc = tc.nc
    f32 = mybir.dt.float32
    MULT = mybir.AluOpType.mult
    ADD = mybir.AluOpType.add
    SIN = mybir.ActivationFunctionType.Sin

    ept = b_core // _P
    U = chunk
    nchunks = ept // U
    assert ept % U == 0

    q_view = q_ap.rearrange("(p e) l -> p e l", p=_P)
    out_view = out_ap.rearrange("(p e) n r c -> p e (n r c)", p=_P)

    with ExitStack() as ctx:
        persist = ctx.enter_context(tc.tile_pool(name="persist", bufs=1))
        scr = ctx.enter_context(tc.tile_pool(name="scr", bufs=1))
        work = scr

        q_sb = persist.tile([_P, ept, 8], f32)
        nc.sync.dma_start(out=q_sb, in_=q_view)
        # B2 constants replicated across partitions: [P, 8, 9]
        b2c = persist.tile([_P, _NL, 9], f32)
        b2_bcast = bass.AP(tensor=b2c_ap.tensor, offset=b2c_ap.offset,
                           ap=[[0, _P], [9, _NL], [1, 9]])
        nc.sync.dma_start(out=b2c, in_=b2_bcast)

        asm_tiles = []
        for bi in range(min(2, nchunks)):
            asm = persist.tile([_P, U, 144], f32, tag=f"asm{bi}")
            a = asm[:]
            nc.vector.memset(_ap(a, 0, [[144, U], [1, 16]]), 0.0)
            nc.vector.memset(_ap(a, 0, [[144, U], [5, 4]]), 1.0)
            nc.gpsimd.memset(_ap(a, 16 + 12, [[144, U], [16, _NL], [1, 3]]), 0.0)
            nc.gpsimd.memset(_ap(a, 16 + 15, [[144, U], [16, _NL]]), 1.0)
            for m in range(3):
                nc.gpsimd.memset(_ap(a, 16 + 4 * m + 3, [[144, U]]), float(TF[0, m]))
            asm_tiles.append(asm)

        for c in range(nchunks):
            asm = asm_tiles[c % 2]
            a = asm[:]
            sl = slice(c * U, (c + 1) * U)

            qs_t = work.tile([_P, _NL, U], f32, tag="X")
            qc_t = work.tile([_P, _NL, U], f32, tag="Y")
            q_chunk = q_sb[:, sl, :].rearrange("p j l -> p l j")
            nc.vector.add_range_wrap(out=qs_t[:], in_=q_chunk, shift=0.0,
                                     bound=_PI, period=2 * _PI)
            nc.vector.add_range_wrap(out=qc_t[:], in_=q_chunk, shift=_PI / 2,
                                     bound=_PI, period=2 * _PI)
            s_t = work.tile([_P, _NL, U], f32, tag="s")
            c_t = work.tile([_P, _NL, U], f32, tag="c")
            nc.scalar.activation(out=s_t[:], in_=qs_t[:], func=SIN)
            nc.scalar.activation(out=c_t[:], in_=qc_t[:], func=SIN)

            for L in range(1, _NL + 1):
                i = L - 1
                boff = L * 16
                poff = (L - 1) * 16

                # ---- Rl on GPSIMD ----
                X = scr.tile([_P, 9, U], f32, tag="X")
                Y = scr.tile([_P, 9, U], f32, tag="Y")
                for mn in range(9):
                    m, n = divmod(mn, 3)
                    nc.gpsimd.tensor_scalar(
                        out=X[:, mn, :], in0=c_t[:, i, :],
                        scalar1=float(B1[i, m, n]), scalar2=float(B0[i, m, n]),
                        op0=MULT, op1=ADD)
                # Y[mn] = s * B2[mn]  (one broadcast TT over all 9 comps)
                s_b = s_t[:, i, :].unsqueeze(1).broadcast_to([_P, 9, U])
                b2_b = b2c[:, i, :].unsqueeze(2).broadcast_to([_P, 9, U])
                nc.gpsimd.tensor_tensor(out=Y[:], in0=s_b, in1=b2_b, op=MULT)
                if L == 1:
                    nc.gpsimd.tensor_tensor(
                        out=_ap(a, boff, [[4, 3], [1, 3], [144, U]]),
                        in0=X[:].rearrange("p (m n) j -> p m n j", m=3),
                        in1=Y[:].rearrange("p (m n) j -> p m n j", m=3), op=ADD)
                    continue
                Rl = scr.tile([_P, 9, U], f32, tag="Rl")
                nc.gpsimd.tensor_tensor(out=Rl[:], in0=X[:], in1=Y[:], op=ADD)

                # ---- t chain (VectorE STT) ----
                acc1 = scr.tile([_P, 3, U], f32, tag="acc1")
                acc2 = scr.tile([_P, 3, U], f32, tag="acc2")
                nc.vector.scalar_tensor_tensor(
                    out=acc1[:], in0=_ap(a, poff + 0, [[4, 3], [144, U]]),
                    scalar=float(TF[i, 0]),
                    in1=_ap(a, poff + 3, [[4, 3], [144, U]]), op0=MULT, op1=ADD)
                nc.vector.scalar_tensor_tensor(
                    out=acc2[:], in0=_ap(a, poff + 1, [[4, 3], [144, U]]),
                    scalar=float(TF[i, 1]), in1=acc1[:], op0=MULT, op1=ADD)
                nc.vector.scalar_tensor_tensor(
                    out=_ap(a, boff + 3, [[4, 3], [144, U]]),
                    in0=_ap(a, poff + 2, [[4, 3], [144, U]]),
                    scalar=float(TF[i, 2]), in1=acc2[:], op0=MULT, op1=ADD)

                # ---- R chain (VectorE): merged mult + 2 adds ----
                prod = scr.tile([_P, 3, 3, 3, U], f32, tag="prod")
                rp_b = _ap(a, poff, [[4, 3], [1, 3], [0, 3], [144, U]])
                rl_b = Rl[:].rearrange("p (k n) j -> p k n j", k=3).unsqueeze(1) \
                    .broadcast_to([_P, 3, 3, 3, U])
                nc.vector.tensor_tensor(out=prod[:], in0=rp_b, in1=rl_b, op=MULT)
                s9 = scr.tile([_P, 3, 3, U], f32, tag="s9")
                nc.vector.tensor_tensor(out=s9[:], in0=prod[:, :, 0], in1=prod[:, :, 1],
                                        op=ADD)
                nc.vector.tensor_tensor(
                    out=_ap(a, boff, [[4, 3], [1, 3], [144, U]]),
                    in0=s9[:], in1=prod[:, :, 2], op=ADD)

            nc.sync.dma_start(out=out_view[:, sl, :], in_=asm)


def trace_fk_v3(tc, out_ap, q_ap, b2c_ap, B0, B1, B2, TF, b_core, chunks,
                act_affine=5, fp16_chain=False, y_on="pool", add_on="pool",
                init_on="pool"):
    """v3: uneven chunks (small tail), link-1 on DVE, double-buffered Rl
    pipeline, affine split between ScalarE (act_affine comps) and GPSIMD.
    fp16_chain: chain state/products in fp16 (DVE 2x), asm writes stay fp32."""
    import concourse.bass as bass
    import concourse.mybir as mybir
    from contextlib import ExitStack

    nc = tc.nc
    f32 = mybir.dt.float32
    f16 = mybir.dt.float16
    cdt = f16 if fp16_chain else f32
    MULT = mybir.AluOpType.mult
    ADD = mybir.AluOpType.add
    COPY = mybir.ActivationFunctionType.Copy
    SIN = mybir.ActivationFunctionType.Sin

    ept = b_core // _P
    assert sum(chunks) == ept
    Umax = max(chunks)

    q_view = q_ap.rearrange("(p e) l -> p e l", p=_P)
    out_view = out_ap.rearrange("(p e) n r c -> p e (n r c)", p=_P)

    with ExitStack() as ctx:
        persist = ctx.enter_context(tc.tile_pool(name="persist", bufs=1))
        scr2 = ctx.enter_context(tc.tile_pool(name="scr2", bufs=2))
        scr1 = ctx.enter_context(tc.tile_pool(name="scr1", bufs=1))

        q_sb = persist.tile([_P, ept, 8], f32)
        nc.sync.dma_start(out=q_sb, in_=q_view)
        b2c = persist.tile([_P, _NL, 9], cdt)
        b2_bcast = bass.AP(tensor=b2c_ap.tensor, offset=b2c_ap.offset,
                           ap=[[0, _P], [9, _NL], [1, 9]])
        if fp16_chain:
            nc.gpsimd.dma_start(out=b2c, in_=b2_bcast)   # SWDGE casts f32->f16
        else:
            nc.sync.dma_start(out=b2c, in_=b2_bcast)

        asm_tiles = []
        for bi in range(2):
            asm = persist.tile([_P, Umax, 144], f32, tag=f"asm{bi}")
            a = asm[:]
            eng_i = nc.gpsimd if init_on == "pool" else nc.vector
            nc.vector.memset(_ap(a, 0, [[144, Umax], [1, 16]]), 0.0)
            nc.vector.memset(_ap(a, 0, [[144, Umax], [5, 4]]), 1.0)
            eng_i.memset(_ap(a, 16 + 12, [[144, Umax], [16, _NL], [1, 3]]), 0.0)
            eng_i.memset(_ap(a, 16 + 15, [[144, Umax], [16, _NL]]), 1.0)
            for m in range(3):
                eng_i.memset(_ap(a, 16 + 4 * m + 3, [[144, Umax]]),
                             float(TF[0, m]))
            asm_tiles.append(asm)

        base = 0
        for c, U in enumerate(chunks):
            asm = asm_tiles[c % 2]
            a = asm[:]
            sl = slice(base, base + U)
            base += U

            # ---- angles ----
            qs_t = scr2.tile([_P, _NL, U], f32, tag="qs")
            qc_t = scr2.tile([_P, _NL, U], f32, tag="qc")
            q_chunk = q_sb[:, sl, :].rearrange("p j l -> p l j")
            nc.vector.add_range_wrap(out=qs_t[:], in_=q_chunk, shift=0.0,
                                     bound=_PI, period=2 * _PI)
            s_t = scr2.tile([_P, _NL, U], cdt, tag="s")
            c_t = scr2.tile([_P, _NL, U], cdt, tag="c")
            if _COS_HALF:
                # cos(q) = 1 - 2 sin^2(q/2); q/2 of wrapped q is in [-pi/2,pi/2]
                # so no second range-wrap is needed. Square on ACT, affine on
                # GPSIMD (both have slack; saves one DVE pass per chunk).
                SQ = mybir.ActivationFunctionType.Square
                nc.scalar.activation(out=s_t[:, 0, :], in_=qs_t[:, 0, :], func=SIN)
                nc.scalar.activation(out=qc_t[:, 0, :], in_=qs_t[:, 0, :],
                                     func=SIN, scale=0.5)
                nc.scalar.activation(out=qc_t[:, 0, :], in_=qc_t[:, 0, :], func=SQ)
                nc.gpsimd.tensor_scalar(out=c_t[:, 0, :], in0=qc_t[:, 0, :],
                                        scalar1=-2.0, scalar2=1.0,
                                        op0=MULT, op1=ADD)
                nc.scalar.activation(out=s_t[:, 1:, :], in_=qs_t[:, 1:, :], func=SIN)
                nc.scalar.activation(out=qc_t[:, 1:, :], in_=qs_t[:, 1:, :],
                                     func=SIN, scale=0.5)
                nc.scalar.activation(out=qc_t[:, 1:, :], in_=qc_t[:, 1:, :], func=SQ)
                nc.gpsimd.tensor_scalar(out=c_t[:, 1:, :], in0=qc_t[:, 1:, :],
                                        scalar1=-2.0, scalar2=1.0,
                                        op0=MULT, op1=ADD)
            else:
                nc.vector.add_range_wrap(out=qc_t[:], in_=q_chunk, shift=_PI / 2,
                                         bound=_PI, period=2 * _PI)
                # link-1 trig first so the pipeline starts immediately
                nc.scalar.activation(out=s_t[:, 0, :], in_=qs_t[:, 0, :], func=SIN)
                nc.scalar.activation(out=c_t[:, 0, :], in_=qc_t[:, 0, :], func=SIN)
                nc.scalar.activation(out=s_t[:, 1:, :], in_=qs_t[:, 1:, :], func=SIN)
                nc.scalar.activation(out=c_t[:, 1:, :], in_=qc_t[:, 1:, :], func=SIN)

            pending_copy = None
            for L in range(1, _NL + 1):
                i = L - 1
                boff = L * 16
                poff = (L - 1) * 16

                if L == 1:
                    # fully on DVE (fills the chunk-start bubble):
                    # asm_R1[mn] = (s*B2 + (c*B1 + B0))
                    X1 = scr2.tile([_P, 9, U], cdt, tag="X1")
                    for mn in range(9):
                        m, n = divmod(mn, 3)
                        nc.vector.tensor_scalar(
                            out=X1[:, mn, :], in0=c_t[:, 0, :],
                            scalar1=float(B1[0, m, n]), scalar2=float(B0[0, m, n]),
                            op0=MULT, op1=ADD)
                    for mn in range(9):
                        m, n = divmod(mn, 3)
                        nc.vector.scalar_tensor_tensor(
                            out=_ap(a, boff + 4 * (mn // 3) + mn % 3, [[144, U]]),
                            in0=s_t[:, 0, :], scalar=float(B2[0, mn // 3, mn % 3]),
                            in1=X1[:, mn, :], op0=MULT, op1=ADD)
                    if fp16_chain:
                        R16p = scr2.tile([_P, 9, U], f16, tag="R16a")
                        nc.scalar.copy(
                            out=R16p[:],
                            in_=_ap(a, boff, [[4, 3], [1, 3], [144, U]]))
                    continue

                # ---- Rl (split: ScalarE affine for some comps, GPSIMD rest) ----
                X = scr2.tile([_P, 9, U], cdt, tag="X")
                Y = scr2.tile([_P, 9, U], cdt, tag="Y")
                if y_on == "aff":
                    # Rl = (c*B1 + B0/2) + (s*B2 + B0/2): 18 per-comp affines,
                    # first act_affine of the 18 on ScalarE, rest on GPSIMD.
                    ops = []
                    for mn in range(9):
                        m, n = divmod(mn, 3)
                        ops.append((X, mn, c_t, float(B1[i, m, n]),
                                    float(B0[i, m, n]) / 2))
                        ops.append((Y, mn, s_t, float(B2[i, m, n]),
                                    float(B0[i, m, n]) / 2))
                    for idx, (dst, mn, src, sc, bi_) in enumerate(ops):
                        if idx < act_affine:
                            nc.scalar.activation(
                                out=dst[:, mn, :], in_=src[:, i, :], func=COPY,
                                scale=sc, bias=bi_)
                        else:
                            nc.gpsimd.tensor_scalar(
                                out=dst[:, mn, :], in0=src[:, i, :],
                                scalar1=sc, scalar2=bi_, op0=MULT, op1=ADD)
                else:
                    for mn in range(9):
                        m, n = divmod(mn, 3)
                        if mn < act_affine:
                            nc.scalar.activation(
                                out=X[:, mn, :], in_=c_t[:, i, :], func=COPY,
                                scale=float(B1[i, m, n]), bias=float(B0[i, m, n]))
                        else:
                            nc.gpsimd.tensor_scalar(
                                out=X[:, mn, :], in0=c_t[:, i, :],
                                scalar1=float(B1[i, m, n]), scalar2=float(B0[i, m, n]),
                                op0=MULT, op1=ADD)
                    if y_on == "dve_ts":
                        for mn in range(9):
                            m, n = divmod(mn, 3)
                            nc.vector.tensor_scalar_mul(
                                out=Y[:, mn, :], in0=s_t[:, i, :],
                                scalar1=float(B2[i, m, n]))
                    else:
                        s_b = s_t[:, i, :].unsqueeze(1).broadcast_to([_P, 9, U])
                        b2_b = b2c[:, i, :].unsqueeze(2).broadcast_to([_P, 9, U])
                        nc.gpsimd.tensor_tensor(out=Y[:], in0=s_b, in1=b2_b, op=MULT)
                Rl = scr2.tile([_P, 9, U], cdt, tag="Rl")
                if add_on == "dve":
                    eng_a = nc.vector
                elif add_on == "split":
                    eng_a = nc.vector if L % 2 else nc.gpsimd
                else:
                    eng_a = nc.gpsimd
                eng_a.tensor_tensor(out=Rl[:], in0=X[:], in1=Y[:], op=ADD)
                if pending_copy is not None:
                    pending_copy()
                    pending_copy = None

                # ---- previous-R access ----
                if fp16_chain:
                    Rp_tile = R16p
                    def rp_k(k, _t=Rp_tile):
                        return _t[:].rearrange("p (m n) j -> p m n j", m=3) \
                            [:, :, k].unsqueeze(2).broadcast_to([_P, 3, 3, U])
                else:
                    def rp_k(k, _a=a, _poff=poff):
                        return _ap(_a, _poff + k, [[4, 3], [0, 3], [144, U]])

                # ---- t chain (fp32; DVE STT, or TS+TT pairs on GPSIMD) ----
                acc1 = scr1.tile([_P, 3, U], f32, tag="acc1")
                acc2 = scr1.tile([_P, 3, U], f32, tag="acc2")
                if fp16_chain:
                    def rp_col(k, _t=Rp_tile):
                        return _t[:].rearrange("p (m n) j -> p m n j", m=3)[:, :, k]
                else:
                    def rp_col(k, _a=a, _poff=poff):
                        return _ap(_a, _poff + k, [[4, 3], [144, U]])
                t_pool = (_T_POOL == "all") or (_T_POOL == "even" and L % 2 == 0)
                if t_pool:
                    u = scr1.tile([_P, 3, U], f32, tag="tu")
                    nc.gpsimd.tensor_scalar(out=u[:], in0=rp_col(0),
                                            scalar1=float(TF[i, 0]), scalar2=None,
                                            op0=MULT)
                    nc.gpsimd.tensor_tensor(
                        out=acc1[:], in0=u[:],
                        in1=_ap(a, poff + 3, [[4, 3], [144, U]]), op=ADD)
                    nc.gpsimd.tensor_scalar(out=u[:], in0=rp_col(1),
                                            scalar1=float(TF[i, 1]), scalar2=None,
                                            op0=MULT)
                    nc.gpsimd.tensor_tensor(out=acc2[:], in0=u[:], in1=acc1[:],
                                            op=ADD)
                    nc.gpsimd.tensor_scalar(out=u[:], in0=rp_col(2),
                                            scalar1=float(TF[i, 2]), scalar2=None,
                                            op0=MULT)
                    nc.gpsimd.tensor_tensor(
                        out=_ap(a, boff + 3, [[4, 3], [144, U]]),
                        in0=u[:], in1=acc2[:], op=ADD)
                else:
                    nc.vector.scalar_tensor_tensor(
                        out=acc1[:], in0=rp_col(0), scalar=float(TF[i, 0]),
                        in1=_ap(a, poff + 3, [[4, 3], [144, U]]), op0=MULT, op1=ADD)
                    nc.vector.scalar_tensor_tensor(
                        out=acc2[:], in0=rp_col(1), scalar=float(TF[i, 1]),
                        in1=acc1[:], op0=MULT, op1=ADD)
                    nc.vector.scalar_tensor_tensor(
                        out=_ap(a, boff + 3, [[4, 3], [144, U]]),
                        in0=rp_col(2), scalar=float(TF[i, 2]), in1=acc2[:],
                        op0=MULT, op1=ADD)

                # ---- R chain (DVE) ----
                def rl_k(k):
                    return Rl[:, 3 * k:3 * k + 3, :].unsqueeze(1).broadcast_to(
                        [_P, 3, 3, U])

                # per-k products (ISA limit: <=3 free dims per AP)
                prod = scr1.tile([_P, 3, 3, 3, U], cdt, tag="prod")
                for k in range(3):
                    nc.vector.tensor_tensor(out=prod[:, :, k], in0=rp_k(k),
                                            in1=rl_k(k), op=MULT)
                s9b = scr1.tile([_P, 3, 3, U], cdt, tag="s9b")
                eng_a1 = nc.gpsimd if (_A1_POOL and L % 2 == 0) else nc.vector
                eng_a1.tensor_tensor(out=s9b[:], in0=prod[:, :, 0],
                                     in1=prod[:, :, 1], op=ADD)
                if fp16_chain:
                    R16n = scr2.tile([_P, 9, U], f16, tag="R16b" if L % 2 else "R16a")
                    nc.vector.tensor_tensor(
                        out=R16n[:].rearrange("p (m n) j -> p m n j", m=3),
                        in0=s9b[:], in1=prod[:, :, 2], op=ADD)

                    def _copy(_R=R16n, _boff=boff):
                        nc.scalar.copy(
                            out=_ap(a, _boff, [[4, 3], [1, 3], [144, U]]),
                            in_=_R[:].rearrange("p (m n) j -> p m n j", m=3))
                    pending_copy = _copy
                    R16p = R16n
                else:
                    nc.vector.tensor_tensor(
                        out=_ap(a, boff, [[4, 3], [1, 3], [144, U]]),
                        in0=s9b[:], in1=prod[:, :, 2], op=ADD)

            if pending_copy is not None:
                pending_copy()
                pending_copy = None
            nc.sync.dma_start(out=out_view[:, sl, :], in_=asm[:, :U, :])


def trace_fk_v4(tc, out_aps, q_ap, B0, B1, B2, TF, b_core,
                x_on="act", y_act_links=0, t_on="pool", cos_half=False,
                rl_pool_links=0, pool_cols=0, pool_scan=False,
                tu_act=False, x_lazy=False):
    """v4: all-fp16 compute, compact component-major tiles [P, comp, U]
    (elem innermost, unit stride => DVE 2x TT / 4x TS modes), only the 93
    non-constant output components DMA'd to HBM as fp16 [8, 12, b_core]
    (host reassembles [B,9,4,4] fp32 and fills constants).

    Engine split: ACT trig + X=c*B1+B0 affines; DVE Y=s*B2 (4x TS),
    Rl=X+Y (2x TT), 3x3 chain products+adds (2x TT); Pool t-chain (STT,
    off critical path) and optionally some Rl adds."""
    import concourse.bass as bass
    import concourse.mybir as mybir
    from contextlib import ExitStack

    nc = tc.nc
    f32 = mybir.dt.float32
    f16 = mybir.dt.float16
    MULT = mybir.AluOpType.mult
    ADD = mybir.AluOpType.add
    BYPASS = mybir.AluOpType.bypass
    COPY = mybir.ActivationFunctionType.Copy
    SIN = mybir.ActivationFunctionType.Sin
    SQ = mybir.ActivationFunctionType.Square

    ept = b_core // _P
    U = ept

    def _t_pool(t_on_):
        return int(t_on_) if str(t_on_).isdigit() else (8 if t_on_ == "pool" else 0)

    q_view = q_ap.rearrange("(p e) l -> p e l", p=_P)   # [P, ept, 8]

    def dram_links(kind, a, b, Wk):
        # per-lane out tensor [P, NL, 12, Wk]: per partition the whole
        # [NL,12,Wk] block is contiguous -> 128 fat descriptors per DMA
        oap = out_aps[0] if kind == "dve" else out_aps[1]
        nl = b - a + 1
        return bass.AP(tensor=oap.tensor,
                       offset=oap.offset + (a - 1) * 12 * Wk,
                       ap=[[_NL * 12 * Wk, _P], [12 * Wk, nl],
                           [Wk, 12], [1, Wk]])

    with ExitStack() as ctx:
        persist = ctx.enter_context(tc.tile_pool(name="persist", bufs=1))
        scr2 = ctx.enter_context(tc.tile_pool(name="scr2", bufs=2))
        scr1 = ctx.enter_context(tc.tile_pool(name="scr1", bufs=1))

        pool_cols = pool_cols * U // 256   # knob calibrated at U=256
        Dd = U - pool_cols
        # ramp pieces: Pool's columns [Dd, U) first so its sub-chain starts
        # as early as possible, then the DVE columns
        pieces = ([(Dd, U - Dd), (0, Dd)] if pool_cols else [(0, U)])

        q_sb = persist.tile([_P, ept, 8], f32)
        for c0, W in pieces:
            nc.sync.dma_start(out=q_sb[:, c0:c0 + W, :],
                              in_=q_view[:, c0:c0 + W, :])

        qlv = q_sb[:].rearrange("p e l -> p l e")        # [P, 8, ept] strided
        qs_t = persist.tile([_P, _NL, U], f32, tag="qs")
        s_t = persist.tile([_P, _NL, U], f16, tag="s")
        c_t = persist.tile([_P, _NL, U], f16, tag="c")
        qc_t = persist.tile([_P, _NL, U], f32, tag="qc")
        u_t = persist.tile([_P, _NL, U], f16, tag="u")
        for c0, W in pieces:
            sl = slice(c0, c0 + W)
            nc.vector.add_range_wrap(out=qs_t[:, :, sl], in_=qlv[:, :, sl],
                                     shift=0.0, bound=_PI, period=2 * _PI)
            nc.scalar.activation(out=s_t[:, :, sl], in_=qs_t[:, :, sl],
                                 func=SIN)
            if cos_half:
                # cos(q) = 1 - 2 sin^2(q/2); wrapped q/2 in [-pi/2, pi/2]
                nc.scalar.activation(out=u_t[:, :, sl], in_=qs_t[:, :, sl],
                                     func=SIN, scale=0.5)
                nc.scalar.activation(out=u_t[:, :, sl], in_=u_t[:, :, sl],
                                     func=SQ)
                nc.vector.tensor_scalar(out=c_t[:, :, sl], in0=u_t[:, :, sl],
                                        scalar1=-2.0, scalar2=1.0,
                                        op0=MULT, op1=ADD)
            else:
                nc.vector.add_range_wrap(out=qc_t[:, :, sl], in_=qlv[:, :, sl],
                                         shift=_PI / 2, bound=_PI,
                                         period=2 * _PI)
                nc.scalar.activation(out=c_t[:, :, sl], in_=qc_t[:, :, sl],
                                     func=SIN)

        # X[i] = c_i*B1[i] + B0[i]  (all links, ahead of the chain)
        Xall = persist.tile([_P, _NL, 9, U], f16, tag="X")

        def emit_x(i, c0, W):
            for mn in range(9):
                m, n = divmod(mn, 3)
                if x_on == "act":
                    nc.scalar.activation(
                        out=Xall[:, i, mn, c0:c0 + W],
                        in_=c_t[:, i, c0:c0 + W], func=COPY,
                        scale=float(B1[i, m, n]), bias=float(B0[i, m, n]))
                else:
                    nc.vector.tensor_scalar(
                        out=Xall[:, i, mn, c0:c0 + W],
                        in0=c_t[:, i, c0:c0 + W],
                        scalar1=float(B1[i, m, n]), scalar2=float(B0[i, m, n]),
                        op0=MULT, op1=ADD)

        for c0, W in pieces:
            emit_x(0, c0, W)
        if not x_lazy:
            for i in range(1, _NL):
                emit_x(i, 0, U)

        Yall = persist.tile([_P, _NL, 9, U], f16, tag="Yall")

        # element-column split: DVE gets [0, D), Pool replays the same
        # algorithm on [D, U) (its own independent sub-chain). Per-lane RT
        # tiles + per-lane HBM tensors: fully disjoint memory, so the two
        # lanes can NEVER false-depend on each other's byte ranges.
        D = Dd
        subs = [("dve", 0, D)]
        if pool_cols:
            subs = [("pool", D, pool_cols)] + subs
        RTd = persist.tile([_P, _NL, 12, D], f16, tag="RTd")
        rt_tiles = {"dve": RTd}
        if pool_cols:
            RTp = persist.tile([_P, _NL, 12, pool_cols], f16, tag="RTp")
            rt_tiles["pool"] = RTp
        lane_w = dict((k, w) for k, _, w in subs)

        def rt_ap(kind, L, comp0, dims):
            rtb = rt_tiles[kind][:]
            Wk = lane_w[kind]
            off = ((L - 1) * 12 + comp0) * Wk
            return bass.AP(tensor=rtb.tensor, offset=rtb.offset + off,
                           ap=[list(rtb.ap[0])] + [list(d) for d in dims])

        # t_1 = tf_1 (constant)
        for kind, c0, W in subs:
            for m in range(3):
                nc.gpsimd.memset(rt_ap(kind, 1, 9 + m, [[1, W]]),
                                 float(TF[0, m]))

        # --- software-pipelined emission: Y/Rl built 2 links ahead so the
        # chain's dependent ops always have independent work in front of
        # them (hides the ~0.5us per-op dependency stall seen on HW) ---
        scr3 = ctx.enter_context(tc.tile_pool(name="scr3", bufs=3))
        rl_tiles = {}

        def emit_y(L):
            i = L - 1
            for yc0, yW in (pieces if L == 1 else [(0, U)]):
                for mn in range(9):
                    m, n = divmod(mn, 3)
                    if i < y_act_links:
                        nc.scalar.activation(
                            out=Yall[:, i, mn, yc0:yc0 + yW],
                            in_=s_t[:, i, yc0:yc0 + yW],
                            func=COPY, scale=float(B2[i, m, n]))
                    else:
                        nc.vector.tensor_scalar_mul(
                            out=Yall[:, i, mn, yc0:yc0 + yW],
                            in0=s_t[:, i, yc0:yc0 + yW],
                            scalar1=float(B2[i, m, n]))

        def emit_rl(L):
            i = L - 1
            for kind, c0, W in subs:
                e = nc.vector if kind == "dve" else nc.gpsimd
                Rl = scr3.tile([_P, 9, W], f16, tag=f"Rl{kind}")
                rl_tiles[(kind, L)] = Rl
                e.tensor_tensor(out=Rl[:], in0=Xall[:, i, :, c0:c0 + W],
                                in1=Yall[:, i, :, c0:c0 + W], op=ADD)

        emit_y(1)
        for kind, c0, W in subs:
            e = nc.vector if kind == "dve" else nc.gpsimd
            e.tensor_tensor(out=rt_ap(kind, 1, 0, [[W, 9], [1, W]]),
                            in0=Xall[:, 0, :, c0:c0 + W],
                            in1=Yall[:, 0, :, c0:c0 + W], op=ADD)
        emit_y(2)
        emit_rl(2)
        emit_y(3)
        emit_rl(3)

        for L in range(2, _NL + 1):
            i = L - 1
            for kind, c0, W in subs:
                eng = nc.vector if kind == "dve" else nc.gpsimd
                sl = slice(c0, c0 + W)
                Rl = rl_tiles.pop((kind, L))
                p = scr1.tile([_P, 27, W], f16, tag=f"p{kind}")
                s9 = scr1.tile([_P, 9, W], f16, tag=f"s9{kind}")
                u = scr1.tile([_P, 3, 3, W], f16, tag=f"tu{kind}")
                a1 = scr1.tile([_P, 3, W], f16, tag=f"a1{kind}")
                a2 = scr1.tile([_P, 3, W], f16, tag=f"a2{kind}")
                pb = p[:]
                rlb = Rl[:]
                # products first (Rl_L finished ~1 link ago; no stall)
                for k in range(3):
                    eng.tensor_tensor(
                        out=bass.AP(tensor=pb.tensor,
                                    offset=pb.offset + 9 * k * W,
                                    ap=[list(pb.ap[0]), [3 * W, 3], [W, 3],
                                        [1, W]]),
                        in0=rt_ap(kind, L - 1, k,
                                  [[3 * W, 3], [0, 3], [1, W]]),
                        in1=bass.AP(tensor=rlb.tensor,
                                    offset=rlb.offset + 3 * k * W,
                                    ap=[list(rlb.ap[0]), [0, 3], [W, 3],
                                        [1, W]]),
                        op=MULT)
                # independent t-chain scalar products fill the p->s9 gap
                for n in range(3):
                    if tu_act:
                        nc.scalar.activation(
                            out=u[:, n],
                            in_=rt_ap(kind, L - 1, n, [[3 * W, 3], [1, W]]),
                            func=COPY, scale=float(TF[i, n]))
                    else:
                        eng.tensor_scalar_mul(
                            out=u[:, n],
                            in0=rt_ap(kind, L - 1, n, [[3 * W, 3], [1, W]]),
                            scalar1=float(TF[i, n]))
                eng.tensor_tensor(out=s9[:], in0=p[:, 0:9], in1=p[:, 9:18],
                                  op=ADD)
                eng.tensor_tensor(out=a1[:], in0=u[:, 0], in1=u[:, 1], op=ADD)
                eng.tensor_tensor(out=a2[:], in0=u[:, 2],
                                  in1=rt_ap(kind, L - 1, 9, [[W, 3], [1, W]]),
                                  op=ADD)
                eng.tensor_tensor(out=rt_ap(kind, L, 0, [[W, 9], [1, W]]),
                                  in0=s9[:], in1=p[:, 18:27], op=ADD)
                eng.tensor_tensor(out=rt_ap(kind, L, 9, [[W, 3], [1, W]]),
                                  in0=a1[:], in1=a2[:], op=ADD)
            if L in (3, 6, _NL):
                a_, b_ = {3: (1, 3), 6: (4, 6), _NL: (7, 8)}[L]
                for kind, c0, W in subs:
                    nc.sync.dma_start(
                        out=dram_links(kind, a_, b_, W),
                        in_=rt_tiles[kind][:, a_ - 1:b_])
            if L + 2 <= _NL:
                emit_y(L + 2)
                emit_rl(L + 2)
            if x_lazy and L < _NL:
                emit_x(L, 0, U)


def assemble_v4(core_outs, TF, core_outs_p=None):
    """core_outs: per-core [P, NL, 12, D] fp16 (DVE lane); core_outs_p:
    per-core [P, NL, 12, pc] (Pool lane cols) -> [B, 9, 4, 4] f32."""
    first = np.asarray(core_outs[0])
    D = first.size // (_NL * _P * 12)
    pc = 0
    if core_outs_p is not None:
        pc = np.asarray(core_outs_p[0]).size // (_NL * _P * 12)
    ept = D + pc
    b_core = _P * ept
    B = b_core * len(core_outs)
    full = np.zeros((B, 9, 4, 4), dtype=np.float32)
    full[:, :, 3, 3] = 1.0
    full[:, 0, 0, 0] = 1.0
    full[:, 0, 1, 1] = 1.0
    full[:, 0, 2, 2] = 1.0
    for ci in range(len(core_outs)):
        a = np.asarray(core_outs[ci]).reshape(_P, _NL, 12, D)
        if pc:
            ap_ = np.asarray(core_outs_p[ci]).reshape(_P, _NL, 12, pc)
            a = np.concatenate([a, ap_], axis=-1)
        a = a.astype(np.float32).transpose(0, 3, 1, 2).reshape(
            b_core, _NL, 12)
        b0 = ci * b_core
        full[b0:b0 + b_core, 1:, :3, :3] = a[:, :, :9].reshape(b_core, _NL, 3, 3)
        full[b0:b0 + b_core, 1:, :3, 3] = a[:, :, 9:12]
    return full


import os as _os
_VARIANT = _os.environ.get("KERNEL_VARIANT", "v4")


def _build_program(B0, B1, B2, TF, b_core, chunk, variant=None, reps=1):
    import concourse.bacc as bacc
    import concourse.mybir as mybir
    import concourse.tile as tile

    variant = variant or _VARIANT
    nc = bacc.Bacc("TRN2", target_bir_lowering=False, debug=False)
    q_d = nc.dram_tensor("q", [b_core, _NL], mybir.dt.float32, kind="ExternalInput")
    if variant == "v4":
        ept = b_core // _P
        pc = _V4_POOL_COLS * ept // 256
        Dd = ept - pc
        out_d = nc.dram_tensor("out", [_P, _NL, 12, Dd], mybir.dt.float16,
                               kind="ExternalOutput")
        outp_d = nc.dram_tensor("outp", [_P, _NL, 12, max(pc, 1)],
                                mybir.dt.float16, kind="ExternalOutput")
        with tile.TileContext(nc) as tc:
            def _body():
                trace_fk_v4(tc, [out_d.ap(), outp_d.ap()], q_d.ap(),
                            B0, B1, B2, TF, b_core,
                            x_on=_V4_X_ON, y_act_links=_V4_Y_ACT,
                            t_on=_V4_T_ON, cos_half=_V4_COS_HALF,
                            rl_pool_links=_V4_RL_POOL, pool_cols=_V4_POOL_COLS,
                            pool_scan=_V4_POOL_SCAN, tu_act=_V4_TU_ACT,
                            x_lazy=_V4_X_LAZY)
            if reps == 1:
                _body()
            else:
                with tc.For_i(0, reps):
                    _body()
        nc.compile()
        return nc
    out_d = nc.dram_tensor("out", [b_core, 9, 4, 4], mybir.dt.float32,
                           kind="ExternalOutput")
    if variant == "v1":
        with tile.TileContext(nc) as tc:
            trace_fk(tc, out_d.ap(), q_d.ap(), B0, B1, B2, TF, b_core, chunk)
    elif variant == "v2":
        b2c_d = nc.dram_tensor("b2c", [_NL * 9], mybir.dt.float32,
                               kind="ExternalInput")
        with tile.TileContext(nc) as tc:
            trace_fk_v2(tc, out_d.ap(), q_d.ap(), b2c_d.ap(), B0, B1, B2, TF,
                        b_core, chunk)
    else:
        b2c_d = nc.dram_tensor("b2c", [_NL * 9], mybir.dt.float32,
                               kind="ExternalInput")
        ept = b_core // _P
        chunks = _mk_chunks(ept)
        fp16 = (variant == "v3fp16")
        aa = _ACT_AFFINE if _ACT_AFFINE is not None else (10 if fp16 else 9)
        yo = _Y_ON if _Y_ON is not None else ("aff" if fp16 else "pool")
        ao = _ADD_ON if _ADD_ON is not None else ("split" if fp16 else "pool")
        with tile.TileContext(nc) as tc:
            def _body3():
                trace_fk_v3(tc, out_d.ap(), q_d.ap(), b2c_d.ap(), B0, B1, B2,
                            TF, b_core, chunks, act_affine=aa, fp16_chain=fp16,
                            y_on=yo, add_on=ao, init_on=_INIT_ON)
            if reps == 1:
                _body3()
            else:
                with tc.For_i(0, reps):
                    _body3()
    nc.compile()
    return nc


_ACT_AFFINE = None
_Y_ON = None
_ADD_ON = None
_INIT_ON = "pool"
_A1_POOL = True
_COS_HALF = False
_T_POOL = "none"
_CHUNKS_FRACS = (0.40625, 0.40625, 0.1875)   # 104/104/48 at ept=256

# v4 knobs
_V4_X_ON = "dve"      # "act" | "dve"
_V4_Y_ACT = 8         # links whose Y=s*B2 runs on ACT instead of DVE
_V4_T_ON = "dve"      # "dve" | "pool" | int (first N links' t on Pool)
_V4_COS_HALF = False  # cos via 1-2sin^2(q/2) (drops 2nd range wrap)
_V4_RL_POOL = 0       # links whose Rl=X+Y add runs on Pool
_V4_POOL_COLS = 48    # element columns handled by a Pool-side sub-chain
_V4_POOL_SCAN = False  # Pool elementwise via tensor_tensor_scan(bypass)
_V4_TU_ACT = True      # t-chain scalar products on ACT (both lanes)
_V4_X_LAZY = False     # BROKEN with 2-ahead Rl pipeline: emit_rl(L+2) reads X before lazy emit_x writes it


def _mk_chunks(ept):
    if ept <= 32:
        return [ept]
    cs = [max(8, int(ept * f) // 8 * 8) for f in _CHUNKS_FRACS[:-1]]
    cs.append(ept - sum(cs))
    assert all(x > 0 for x in cs)
    return cs


def kernel(q, fixed_rot, fixed_trans, joint_axis):
    from concourse import bass_utils

    q = np.asarray(q, dtype=np.float32)
    B = q.shape[0]
    b_core = B // _N_CORES
    B0, B1, B2, TF = _fk_constants(np.asarray(fixed_rot), np.asarray(fixed_trans),
                                   np.asarray(joint_axis))
    nc = _build_program(B0, B1, B2, TF, b_core, _U)
    in_maps = [{"q": np.ascontiguousarray(q[i * b_core:(i + 1) * b_core])}
               for i in range(_N_CORES)]
    if _VARIANT not in ("v1", "v4"):
        b2c = np.ascontiguousarray(B2.reshape(-1).astype(np.float32))
        for m in in_maps:
            m["b2c"] = b2c
    res = bass_utils.run_bass_kernel_spmd(nc, in_maps, core_ids=list(range(_N_CORES)))
    if _VARIANT == "v4":
        return assemble_v4([res.results[i]["out"] for i in range(_N_CORES)],
                           TF,
                           [res.results[i]["outp"] for i in range(_N_CORES)]
                           if _V4_POOL_COLS else None)
    out = np.concatenate([res.results[i]["out"] for i in range(_N_CORES)], axis=0)
    return out.astype(np.float32)

